# revision 1
# baseline (speedup 1.0000x reference)
"""Distributed Trainium2 kernel for 3-layer GraphConv GNN + global mean pool + L2 normalize.

Strategy (8 NeuronCores, SPMD):
  - Nodes sharded by contiguous ranges across cores (dst-sharding of edges).
  - Aggregation (segment_sum of gathered neighbor features) per core:
      * dma_gather pulls h[src] rows from a replicated node-feature table in HBM
        (int16 index limit handled by splitting the table into 32768-row blocks).
      * scatter side is a one-hot matmul into PSUM: for each chunk of <=128 edges,
        PSUM[tile] += onehot(dst_slot)^T-style matmul. Exact f32 accumulation.
  - Dense phase per layer on TensorEngine (bf16 operands, f32 PSUM).
  - h replicated between layers with collective AllGather (bf16).
  - Global mean-pool via batch-one-hot matmul, AllReduce of [G, 512] partials,
    then L2 normalization. All cores produce the full output.

Host-side work is strictly index preprocessing (sorting/partitioning per the
METIS-style sharding hint); no float input values are touched on host.
"""

import math
import sys

import numpy as np

sys.path.insert(0, "/opt/trn_rl_repo")

import ml_dtypes  # noqa: E402

BF16 = ml_dtypes.bfloat16

# ----------------------------------------------------------------------------
# Configs
# ----------------------------------------------------------------------------

FULL_CFG = dict(N=100000, E=800000, G=64, NC=8)
DIMS = [1, 128, 256, 512]
SUPER = 8        # dst tiles per super-iteration (= PSUM banks used)
PADQ = 128        # per-(super,block,tile) segment padding quantum
BLK = 32768      # int16 index block size for gather tables
WIN = 64         # layer-1 f32 gather window (256B)
GCAP = 1024      # max indices per dma_gather call (SWDGE ring limit)
SL = 2048        # edges per SBUF slice (gather/one-hot staging)


def derive(cfg):
    d = dict(cfg)
    N, NC = d["N"], d["NC"]
    assert N % NC == 0
    d["NPC"] = N // NC
    d["TPC"] = (d["NPC"] + 127) // 128          # node tiles per core
    d["NPC_PAD"] = d["TPC"] * 128
    d["NFULL"] = NC * d["NPC_PAD"]
    d["NBLK"] = (d["NFULL"] + BLK - 1) // BLK
    d["NSUP"] = (d["TPC"] + SUPER - 1) // SUPER
    d["NW1"] = (d["N"] + WIN - 1) // WIN        # x windows
    return d


# ----------------------------------------------------------------------------
# Host preprocessing: edge layout + schedule
# ----------------------------------------------------------------------------

def preprocess(x, edge_index, batch, cfg):
    """Build all per-core host arrays and the static schedule."""
    c = cfg
    N, E, G, NC = c["N"], c["E"], c["G"], c["NC"]
    NPC, TPC, NPC_PAD, NFULL, NBLK, NSUP = (
        c["NPC"], c["TPC"], c["NPC_PAD"], c["NFULL"], c["NBLK"], c["NSUP"])

    src = np.asarray(edge_index[0], dtype=np.int64)
    dst = np.asarray(edge_index[1], dtype=np.int64)
    batch = np.asarray(batch, dtype=np.int64)

    # ---- per-core edge sets
    core_of = dst // NPC
    per_core = []
    for ci in range(NC):
        m = core_of == ci
        es, ed = src[m], dst[m] - ci * NPC
        per_core.append((es, ed))

    # ---- G: main gather layout (shared by layers 2 and 3)
    # order: (super, block, tile, src)
    def g_keys(es, ed):
        tile = ed >> 7
        slot = ed & 127
        sup = tile // SUPER
        spad = (es // NPC) * NPC_PAD + (es % NPC)
        blk = spad // BLK
        return sup, blk, tile, slot, spad

    # segment counts n[core, sup, blk, tile]
    nseg = np.zeros((NC, NSUP, NBLK, TPC), dtype=np.int64)
    gdata = []
    for ci in range(NC):
        es, ed = per_core[ci]
        sup, blk, tile, slot, spad = g_keys(es, ed)
        order = np.lexsort((spad, tile, blk, sup))
        sup, blk, tile, slot, spad = (a[order] for a in (sup, blk, tile, slot, spad))
        np.add.at(nseg[ci], (sup, blk, tile), 1)
        gdata.append((sup, blk, tile, slot, spad))

    nmax = nseg.max(axis=0)  # [NSUP, NBLK, TPC]
    npad = ((nmax + PADQ - 1) // PADQ) * PADQ
    # ensure every (sup, tile) has at least one segment so PSUM gets written
    tile_tot = npad.sum(axis=1)  # [NSUP, TPC]
    for s in range(NSUP):
        for t in range(min(TPC - s * SUPER, SUPER)):
            ti = s * SUPER + t
            if ti < TPC and tile_tot[s, ti] == 0:
                npad[s, 0, ti] = PADQ

    # run = (sup, blk). run length padded to 128.
    run_len = {}
    run_off = {}   # global edge offset of run start
    seg_off = {}   # (s,b,t) -> global offset
    LT = 0
    for s in range(NSUP):
        for b in range(NBLK):
            r0 = LT
            for t in range(TPC):
                if npad[s, b, t]:
                    seg_off[(s, b, t)] = LT
                    LT += int(npad[s, b, t])
            L = LT - r0
            Lp = ((L + 127) // 128) * 128
            LT = r0 + Lp
            run_len[(s, b)] = Lp
            run_off[(s, b)] = r0
    LTG = LT

    # per-core arrays: gidx int16 (block-local padded src), slotG bf16
    gidx = np.zeros((NC, LTG), dtype=np.int16)
    slotG = np.full((NC, LTG), -1.0, dtype=np.float32)
    for ci in range(NC):
        sup, blk, tile, slot, spad = gdata[ci]
        # fill per segment
        pos = 0
        # edges are sorted by (sup, blk, tile); walk segments
        seg_ids = sup * (NBLK * TPC) + blk * TPC + tile
        bounds = np.flatnonzero(np.diff(seg_ids)) + 1
        starts = np.concatenate(([0], bounds))
        ends = np.concatenate((bounds, [len(seg_ids)]))
        for st, en in zip(starts, ends):
            s, b, t = int(sup[st]), int(blk[st]), int(tile[st])
            o = seg_off[(s, b, t)]
            n = en - st
            assert n <= npad[s, b, t]
            loc = spad[st:en] - b * BLK
            assert (loc >= 0).all() and (loc < BLK).all()
            gidx[ci, o:o + n] = loc.astype(np.int16)
            slotG[ci, o:o + n] = slot[st:en].astype(np.float32)
            # pad entries within segment: repeat first idx (slot stays -1)
            gidx[ci, o + n: o + int(npad[s, b, t])] = loc[0] if n else 0
        del pos

    # slice-centric schedule: per s: tiles + runs; each run split into slices
    # of <= SL edges; pieces attached to the slice containing their column.
    SLC = SL // 128
    sched_g = []
    for s in range(NSUP):
        tiles = list(range(s * SUPER, min((s + 1) * SUPER, TPC)))
        # pieces per tile in edge order, with start/stop flags
        runs = []
        for b in range(NBLK):
            L = run_len[(s, b)]
            if not L:
                continue
            ncols = L // 128
            slices = []
            for c0 in range(0, ncols, SLC):
                nc_ = min(SLC, ncols - c0)
                slices.append(dict(c0=c0, ncols=nc_,
                                   off=run_off[(s, b)] + c0 * 128,
                                   num=nc_ * 128, pieces=[]))
            runs.append(dict(b=b, off=run_off[(s, b)], num=L, slices=slices))
        run_by_b = {r["b"]: r for r in runs}
        for t in tiles:
            pieces = []
            for b in range(NBLK):
                if (s, b, t) not in seg_off:
                    continue
                o = seg_off[(s, b, t)]
                ln_tot = int(npad[s, b, t])
                lo = o - run_off[(s, b)]
                while ln_tot > 0:
                    p0 = lo % 128
                    cap = 128 if p0 == 0 else (64 if p0 == 64 else 32)
                    l = min(ln_tot, cap)
                    pieces.append((b, lo // 128, p0, l))
                    lo += l
                    ln_tot -= l
            assert pieces
            for i, (b, col, p0, l) in enumerate(pieces):
                sl = run_by_b[b]["slices"][col // SLC]
                sl["pieces"].append(dict(
                    t=t, col=col - sl["c0"], p0=p0, ln=l,
                    start=(i == 0), stop=(i == len(pieces) - 1)))
        sched_g.append(dict(tiles=tiles, runs=runs))

    # ---- W: layer-1 gather layout: order (super, tile, src)
    def w_keys(es, ed):
        tile = ed >> 7
        slot = ed & 127
        sup = tile // SUPER
        win = es // WIN
        off = es % WIN
        return sup, tile, slot, win, off

    nseg1 = np.zeros((NC, NSUP, TPC), dtype=np.int64)
    wdata = []
    for ci in range(NC):
        es, ed = per_core[ci]
        sup, tile, slot, win, off = w_keys(es, ed)
        order = np.lexsort((win, tile, sup))
        sup, tile, slot, win, off = (a[order] for a in (sup, tile, slot, win, off))
        np.add.at(nseg1[ci], (sup, tile), 1)
        wdata.append((sup, tile, slot, win, off))

    nmax1 = nseg1.max(axis=0)
    npad1 = ((nmax1 + PADQ - 1) // PADQ) * PADQ
    for s in range(NSUP):
        for t in range(s * SUPER, min((s + 1) * SUPER, TPC)):
            if npad1[s, t] == 0:
                npad1[s, t] = PADQ

    seg_off1 = {}
    run_len1 = {}
    run_off1 = {}
    LT = 0
    for s in range(NSUP):
        r0 = LT
        for t in range(TPC):
            if npad1[s, t]:
                seg_off1[(s, t)] = LT
                LT += int(npad1[s, t])
        L = LT - r0
        Lp = ((L + 127) // 128) * 128
        LT = r0 + Lp
        run_len1[s] = Lp
        run_off1[s] = r0
    LT1 = LT

    widx = np.zeros((NC, LT1), dtype=np.int16)
    woff = np.full((NC, LT1), -1.0, dtype=np.float32)
    slot1 = np.full((NC, LT1), -1.0, dtype=np.float32)
    for ci in range(NC):
        sup, tile, slot, win, off = wdata[ci]
        seg_ids = sup * TPC + tile
        bounds = np.flatnonzero(np.diff(seg_ids)) + 1
        starts = np.concatenate(([0], bounds))
        ends = np.concatenate((bounds, [len(seg_ids)]))
        for st, en in zip(starts, ends):
            s, t = int(sup[st]), int(tile[st])
            o = seg_off1[(s, t)]
            n = en - st
            widx[ci, o:o + n] = win[st:en].astype(np.int16)
            woff[ci, o:o + n] = off[st:en].astype(np.float32)
            slot1[ci, o:o + n] = slot[st:en].astype(np.float32)
            widx[ci, o + n:o + int(npad1[s, t])] = win[0] if n else 0

    sched_1 = []
    for s in range(NSUP):
        tiles = list(range(s * SUPER, min((s + 1) * SUPER, TPC)))
        L = run_len1[s]
        ncols = L // 128
        slices = []
        for c0 in range(0, ncols, SLC):
            nc_ = min(SLC, ncols - c0)
            slices.append(dict(c0=c0, ncols=nc_, off=run_off1[s] + c0 * 128,
                               num=nc_ * 128, pieces=[]))
        for t in tiles:
            pieces = []
            if (s, t) in seg_off1:
                o = seg_off1[(s, t)]
                ln_tot = int(npad1[s, t])
                lo = o - run_off1[s]
                while ln_tot > 0:
                    p0 = lo % 128
                    cap = 128 if p0 == 0 else (64 if p0 == 64 else 32)
                    l = min(ln_tot, cap)
                    pieces.append((lo // 128, p0, l))
                    lo += l
                    ln_tot -= l
            assert pieces
            for i, (col, p0, l) in enumerate(pieces):
                sl = slices[col // SLC]
                sl["pieces"].append(dict(
                    t=t, col=col - sl["c0"], p0=p0, ln=l,
                    start=(i == 0), stop=(i == len(pieces) - 1)))
        sched_1.append(dict(tiles=tiles, slices=slices))

    # ---- idx wrap helper: entry i -> [i%16 (+16g), i//16], replicated 8 groups
    def wrap16(a):
        # a: [NC, L] -> [NC, 128, L//16]
        L = a.shape[1]
        assert L % 16 == 0
        w = a.reshape(a.shape[0], L // 16, 16).transpose(0, 2, 1)  # [NC,16,L/16]
        return np.tile(w, (1, 8, 1)).copy()

    def wrap128(a, dtype):
        L = a.shape[1]
        assert L % 128 == 0
        return a.reshape(a.shape[0], L // 128, 128).transpose(0, 2, 1).astype(dtype).copy()

    host = {}
    host["gidx"] = wrap16(gidx)                       # [NC,128,LTG/16] i16
    host["slotG"] = wrap128(slotG, BF16)              # [NC,128,LTG/128]
    host["widx"] = wrap16(widx)                       # [NC,128,LT1/16]
    host["woff"] = wrap128(woff, np.float32)
    host["slot1"] = wrap128(slot1, BF16)

    # ---- x windows, x local, batch slots, counts
    xf = np.asarray(x, dtype=np.float32).reshape(-1)
    xw = np.zeros((c["NW1"] * WIN,), dtype=np.float32)
    xw[:N] = xf
    host["xw"] = xw.reshape(c["NW1"], WIN)

    xloc = np.zeros((NC, 1, NPC_PAD), dtype=np.float32)
    bslot = np.full((NC, NPC_PAD), -1.0, dtype=np.float32)
    for ci in range(NC):
        xloc[ci, 0, :NPC] = xf[ci * NPC:(ci + 1) * NPC]
        bslot[ci, :NPC] = batch[ci * NPC:(ci + 1) * NPC].astype(np.float32)
    host["xloc"] = xloc.astype(BF16)
    # bslot as [128, TPC]: node 128*t+p -> [p, t]
    host["bslot"] = bslot.reshape(NC, TPC, 128).transpose(0, 2, 1).astype(np.float32).copy()

    counts = np.bincount(batch, minlength=G).astype(np.float64)
    host["invcnt"] = (1.0 / np.maximum(counts, 1.0)).astype(np.float32).reshape(G, 1)

    host["onesrow"] = np.ones((1, NPC_PAD), dtype=BF16)
    host["ident"] = np.eye(128, dtype=np.float32).astype(BF16)
    host["iota128"] = np.broadcast_to(
        np.arange(128, dtype=np.float32), (128, 128)).astype(BF16).copy()
    host["iota64f"] = np.broadcast_to(
        np.arange(WIN, dtype=np.float32), (128, WIN)).copy()
    host["iotaGb"] = np.broadcast_to(
        np.arange(G, dtype=np.float32), (128, G)).astype(BF16).copy()
    host["onesb"] = np.ones((1, 128), dtype=np.float32).astype(BF16)

    sched = dict(sched_g=sched_g, sched_1=sched_1, LTG=LTG, LT1=LT1)
    return host, sched


# ----------------------------------------------------------------------------
# Graph builder
# ----------------------------------------------------------------------------

def build_graph(cfg, sched, debug=False, dump=False):
    from concourse import bass, bacc, tile, mybir

    c = cfg
    G = c["G"]
    NPC_PAD, NFULL, TPC, NSUP = c["NPC_PAD"], c["NFULL"], c["TPC"], c["NSUP"]
    f32 = mybir.dt.float32
    bf16 = mybir.dt.bfloat16
    fp8 = mybir.dt.float8e4
    i16 = mybir.dt.int16
    AF = mybir.ActivationFunctionType
    ALU = mybir.AluOpType

    LTG, LT1 = sched["LTG"], sched["LT1"]

    nc = bacc.Bacc("TRN2", target_bir_lowering=False, debug=debug,
                   num_devices=c["NC"], num_swdge_queues=4)

    # ---------------- dram parameters ----------------
    def din(name, shape, dtype):
        return nc.dram_tensor(name, list(shape), dtype, kind="ExternalInput")

    p = {}
    p["xw"] = din("xw", (c["NW1"], WIN), f32)
    p["xloc"] = din("xloc", (1, NPC_PAD), bf16)
    p["onesrow"] = din("onesrow", (1, NPC_PAD), bf16)
    p["w1stack"] = din("w1stack", (3, 128), bf16)
    p["wrel2"] = din("wrel2", (128, 256), f32)
    p["wroot2"] = din("wroot2", (128, 256), f32)
    p["b2"] = din("b2", (1, 256), f32)
    p["wrel3"] = din("wrel3", (256, 512), f32)
    p["wroot3"] = din("wroot3", (256, 512), f32)
    p["b3"] = din("b3", (1, 512), f32)
    p["ident"] = din("ident", (128, 128), bf16)
    p["iota128"] = din("iota128", (128, 128), bf16)
    p["iota64f"] = din("iota64f", (128, WIN), f32)
    p["iotaGb"] = din("iotaGb", (128, G), bf16)
    p["onesb"] = din("onesb", (1, 128), bf16)
    p["invcnt"] = din("invcnt", (G, 1), f32)
    p["bslot"] = din("bslot", (128, TPC), f32)
    p["widx"] = din("widx", (128, LT1 // 16), i16)
    p["woff"] = din("woff", (128, LT1 // 128), f32)
    p["slot1"] = din("slot1", (128, LT1 // 128), bf16)
    p["gidx"] = din("gidx", (128, LTG // 16), i16)
    p["slotG"] = din("slotG", (128, LTG // 128), bf16)

    out_ext = nc.dram_tensor("out", [G, 512], f32, kind="ExternalOutput")
    if dump:
        dbg_h1 = nc.dram_tensor("dbg_h1", [NFULL, 128], bf16, kind="ExternalOutput")
        dbg_h2 = nc.dram_tensor("dbg_h2", [NFULL, 256], bf16, kind="ExternalOutput")
        dbg_agg1 = nc.dram_tensor("dbg_agg1", [3, NPC_PAD], f32, kind="ExternalOutput")
        dbg_pool = nc.dram_tensor("dbg_pool", [G, 512], f32, kind="ExternalOutput")

    # internal dram
    h1_mine = nc.dram_tensor("h1_mine", [NPC_PAD, 128], bf16)
    h1_full = nc.dram_tensor("h1_full", [NFULL, 128], bf16, addr_space="Shared")
    h2_mine = nc.dram_tensor("h2_mine", [NPC_PAD, 256], bf16)
    h2_full = nc.dram_tensor("h2_full", [NFULL, 256], bf16, addr_space="Shared")
    pool_in = nc.dram_tensor("pool_in", [G, 512], f32)
    pool_out8 = nc.dram_tensor("pool_out8", [8 * G, 512], f32, addr_space="Shared")

    # ---------------- persistent sbuf ----------------
    # arena: h1T / agg2T during L1-L2; agg3 (node-major [128, TPC*256]) in L3
    arena = nc.alloc_sbuf_tensor("arena", [128, 2 * NPC_PAD], bf16)
    h1T = arena.ap()[:, 0:NPC_PAD]
    agg2T = arena.ap()[:, NPC_PAD:2 * NPC_PAD]
    agg3 = arena.ap().rearrange("p (t d) -> p t d", d=256)  # [128, ..., 256]

    h2T0 = nc.alloc_sbuf_tensor("h2T0", [128, NPC_PAD], bf16)
    h2T1 = nc.alloc_sbuf_tensor("h2T1", [128, NPC_PAD], bf16)
    stack3 = nc.alloc_sbuf_tensor("stack3", [3, NPC_PAD], bf16)
    pooled_acc = nc.alloc_sbuf_tensor("pooled_acc", [G, 512], f32)

    ws = {}
    for name, shape, dt_ in [
        ("w1stack", (3, 128), bf16), ("ident", (128, 128), bf16),
        ("iota128", (128, 128), bf16), ("iota64f", (128, WIN), f32),
        ("iotaGb", (128, G), bf16), ("onesb", (1, 128), bf16),
        ("invcnt", (G, 1), f32), ("bslot", (128, TPC), f32),
    ]:
        ws[name] = nc.alloc_sbuf_tensor("sb_" + name, list(shape), dt_)
    # bf16 weights
    wsb = {}
    for name, shape in [("wrel2", (128, 256)), ("wroot2", (128, 256)),
                        ("b2", (1, 256)), ("b3", (1, 512))]:
        wsb[name] = nc.alloc_sbuf_tensor("sbb_" + name, list(shape), bf16)
    for name in ("wrel3", "wroot3"):
        wsb[name + "_0"] = nc.alloc_sbuf_tensor("sbb_" + name + "_0", [128, 512], bf16)
        wsb[name + "_1"] = nc.alloc_sbuf_tensor("sbb_" + name + "_1", [128, 512], bf16)

    with tile.TileContext(nc) as tc:
        # ---------------- load constants ----------------
        with tc.tile_pool(name="wtmp", bufs=2) as wtmp:
            for name in ("w1stack", "ident", "iota128", "iota64f", "iotaGb",
                         "onesb", "invcnt", "bslot"):
                nc.sync.dma_start(ws[name].ap(), p[name].ap())
            for name in ("wrel2", "wroot2", "b2", "b3"):
                t = wtmp.tile(list(p[name].shape), f32, tag="wtmp")
                nc.sync.dma_start(t[:], p[name].ap())
                nc.scalar.copy(wsb[name].ap(), t[:])
            for name in ("wrel3", "wroot3"):
                for k in range(2):
                    t = wtmp.tile([128, 512], f32, tag="wtmp3")
                    nc.sync.dma_start(t[:], p[name].ap()[k * 128:(k + 1) * 128, :])
                    nc.scalar.copy(wsb[name + f"_{k}"].ap(), t[:])
            nc.sync.dma_start(stack3.ap()[1:2, :], p["xloc"].ap())
            nc.sync.dma_start(stack3.ap()[2:3, :], p["onesrow"].ap())

        scope_l1agg = nc.named_scope("l1agg"); scope_l1agg.__enter__()
        # ================= LAYER 1 aggregation =================
        # gather x windows; v = sum(Xg * (iota==off)); psum[1,128] += v^T onehot
        with tc.tile_pool(name="g1", bufs=4) as gpool, \
             tc.tile_pool(name="i1", bufs=6) as ipool, \
             tc.tile_pool(name="s1", bufs=6) as spool, \
             tc.tile_pool(name="p1", bufs=SUPER, space="PSUM") as ppool, \
             tc.tile_pool(name="m1", bufs=4) as mpool:
            for s_ent in sched["sched_1"]:
                pts = {}
                for t in s_ent["tiles"]:
                    pts[t] = ppool.tile([1, 128], f32, tag="ps", name=f"ps1_{t}")
                for sl in s_ent["slices"]:
                    off, num, C = sl["off"], sl["num"], sl["ncols"]
                    idx_t = ipool.tile([128, num // 16], i16, tag="idx")
                    nc.sync.dma_start(idx_t[:], p["widx"].ap()[:, off // 16:(off + num) // 16])
                    xg = gpool.tile([128, C, WIN], f32, tag="g")
                    for e0 in range(0, num, GCAP):
                        n = min(GCAP, num - e0)
                        nc.gpsimd.dma_gather(
                            xg[:, e0 // 128:(e0 + n) // 128, :], p["xw"].ap(),
                            idx_t[:, e0 // 16:(e0 + n) // 16], n, n, WIN,
                            queue_num=(sl["off"] + e0) // GCAP % 4)
                    offc = off // 128
                    woff_t = spool.tile([128, C], f32, tag="woff")
                    nc.scalar.dma_start(woff_t[:], p["woff"].ap()[:, offc:offc + C])
                    slot_t = spool.tile([128, C], bf16, tag="slot")
                    nc.scalar.dma_start(slot_t[:], p["slot1"].ap()[:, offc:offc + C])
                    # mask / v
                    mask = mpool.tile([128, C, WIN], f32, tag="mask")
                    iota_b = ws["iota64f"].ap().rearrange("p w -> p () w").broadcast_to((128, C, WIN))
                    woff_b = woff_t[:].rearrange("p c -> p c ()").broadcast_to((128, C, WIN))
                    nc.vector.tensor_tensor(mask[:], iota_b, woff_b, ALU.is_equal)
                    nc.vector.tensor_tensor(mask[:], mask[:], xg[:], ALU.mult)
                    vf = mpool.tile([128, C], f32, tag="vf")
                    nc.vector.tensor_reduce(vf[:], mask[:], mybir.AxisListType.X, ALU.add)
                    vb = mpool.tile([128, C], bf16, tag="vb")
                    nc.scalar.copy(vb[:], vf[:])
                    # S one-hot
                    S = spool.tile([128, C, 128], bf16, tag="S")
                    iota_s = ws["iota128"].ap().rearrange("p f -> p () f").broadcast_to((128, C, 128))
                    slot_b = slot_t[:].rearrange("p c -> p c ()").broadcast_to((128, C, 128))
                    nc.vector.tensor_tensor(S[:], iota_s, slot_b, ALU.is_equal)
                    for pc in sl["pieces"]:
                        t, col, p0, l = pc["t"], pc["col"], pc["p0"], pc["ln"]
                        nc.tensor.matmul(
                            pts[t][:],
                            vb[p0:p0 + l, col:col + 1],
                            S[p0:p0 + l, col, :],
                            start=pc["start"], stop=pc["stop"])
                for t in s_ent["tiles"]:
                    nc.scalar.copy(stack3.ap()[0:1, t * 128:(t + 1) * 128], pts[t][:])

        scope_l1agg.__exit__(None, None, None)
        scope_l1d = nc.named_scope("l1dense"); scope_l1d.__enter__()
        # ================= LAYER 1 dense =================
        with tc.tile_pool(name="d1p", bufs=3, space="PSUM") as dpsum, \
             tc.tile_pool(name="d1s", bufs=3) as dsb, \
             tc.tile_pool(name="t1p", bufs=2, space="PSUM") as tpsum:
            for t in range(TPC):
                zt = dpsum.tile([128, 128], f32, tag="z")
                nc.tensor.matmul(zt[:], stack3.ap()[:, t * 128:(t + 1) * 128],
                                 ws["w1stack"].ap(), start=True, stop=True)
                ht = dsb.tile([128, 128], bf16, tag="h")
                nc.scalar.activation(ht[:], zt[:], AF.Relu)
                nc.sync.dma_start(h1_mine.ap()[t * 128:(t + 1) * 128, :], ht[:])
                tp = tpsum.tile([128, 128], bf16, tag="tp")
                nc.tensor.transpose(tp[:], ht[:], ws["ident"].ap())
                nc.scalar.copy(h1T[:, t * 128:(t + 1) * 128], tp[:])

        scope_l1d.__exit__(None, None, None)
        scope_x1 = nc.named_scope("xchg1"); scope_x1.__enter__()
        # ================= exchange h1 =================
        nc.gpsimd.collective_compute(
            "AllGather", ALU.bypass, replica_groups=[list(range(c["NC"]))],
            ins=[h1_mine.ap().opt()], outs=[h1_full.ap().opt()])

        # ================= generic aggregation for layers 2/3 =============
        def agg_layer(h_full, d_in, out_write, xg_stationary, xg_dt=bf16):
            """out_write(t, psum_ap): evacuate tile t's psum.

            xg_stationary=True: psum[d_in, 128dst] (lhsT=Xg) — used for L2 so
            the evac lands directly in feature-major agg2T.
            xg_stationary=False: psum[128dst, d_in] (lhsT=S) — used for L3.
            """
            elem = d_in  # bf16 elements per row
            with tc.tile_pool(name="gA", bufs=4) as gpool, \
                 tc.tile_pool(name="iA", bufs=6) as ipool, \
                 tc.tile_pool(name="sA", bufs=6) as spool, \
                 tc.tile_pool(name="pA", bufs=SUPER, space="PSUM") as ppool:
                shape = [d_in, 128] if xg_stationary else [128, d_in]
                for s_ent in sched["sched_g"]:
                    pts = {}
                    for t in s_ent["tiles"]:
                        pts[t] = ppool.tile(shape, f32, tag="ps", name=f"psA_{t}")
                    for run in s_ent["runs"]:
                        b = run["b"]
                        blk_rows = min(BLK, NFULL - b * BLK)
                        for sl in run["slices"]:
                            off, num, C = sl["off"], sl["num"], sl["ncols"]
                            idx_t = ipool.tile([128, num // 16], i16, tag="idx")
                            nc.sync.dma_start(
                                idx_t[:], p["gidx"].ap()[:, off // 16:(off + num) // 16])
                            xg = gpool.tile([128, C, elem], xg_dt, tag="g")
                            for e0 in range(0, num, GCAP):
                                n = min(GCAP, num - e0)
                                nc.gpsimd.dma_gather(
                                    xg[:, e0 // 128:(e0 + n) // 128, :],
                                    h_full.ap()[b * BLK:b * BLK + blk_rows, :],
                                    idx_t[:, e0 // 16:(e0 + n) // 16], n, n, elem,
                                    queue_num=(sl["off"] + e0) // GCAP % 4)
                            offc = off // 128
                            slot_t = spool.tile([128, C], bf16, tag="slot")
                            nc.scalar.dma_start(slot_t[:], p["slotG"].ap()[:, offc:offc + C])
                            S = spool.tile([128, C, 128], bf16, tag="S")
                            iota_s = ws["iota128"].ap().rearrange("p f -> p () f").broadcast_to((128, C, 128))
                            slot_b = slot_t[:].rearrange("p c -> p c ()").broadcast_to((128, C, 128))
                            nc.vector.tensor_tensor(S[:], iota_s, slot_b, ALU.is_equal)
                            for pc in sl["pieces"]:
                                t, col, p0, l = pc["t"], pc["col"], pc["p0"], pc["ln"]
                                if xg_stationary:
                                    lhsT, rhs = xg[p0:p0 + l, col, :], S[p0:p0 + l, col, :]
                                else:
                                    lhsT, rhs = S[p0:p0 + l, col, :], xg[p0:p0 + l, col, :]
                                nc.tensor.matmul(
                                    pts[t][:], lhsT, rhs,
                                    start=pc["start"], stop=pc["stop"])
                    for t in s_ent["tiles"]:
                        out_write(t, pts[t])

        scope_x1.__exit__(None, None, None)
        scope_l2a = nc.named_scope("l2agg"); scope_l2a.__enter__()
        # ---- layer 2: psum [128din, 128dst] -> agg2T directly
        def l2_write(t, pt):
            nc.scalar.copy(agg2T[:, t * 128:(t + 1) * 128], pt[:])
        agg_layer(h1_full, 128, l2_write, xg_stationary=True)

        scope_l2a.__exit__(None, None, None)
        scope_l2d = nc.named_scope("l2dense"); scope_l2d.__enter__()
        # ================= LAYER 2 dense =================
        with tc.tile_pool(name="d2p", bufs=3, space="PSUM") as dpsum, \
             tc.tile_pool(name="d2s", bufs=4) as dsb, \
             tc.tile_pool(name="t2pp", bufs=3, space="PSUM") as tpsum:
            for t in range(TPC):
                cols = slice(t * 128, (t + 1) * 128)
                zt = dpsum.tile([128, 256], f32, tag="z")
                nc.tensor.matmul(zt[:], agg2T[:, cols], wsb["wrel2"].ap(), start=True, stop=False)
                nc.tensor.matmul(zt[:], h1T[:, cols], wsb["wroot2"].ap(), start=False, stop=False)
                nc.tensor.matmul(zt[:], ws["onesb"].ap(), wsb["b2"].ap(), start=False, stop=True)
                ht = dsb.tile([128, 256], bf16, tag="h")
                nc.scalar.activation(ht[:], zt[:], AF.Relu)
                nc.sync.dma_start(h2_mine.ap()[t * 128:(t + 1) * 128, :], ht[:])
                for k in range(2):
                    tp = tpsum.tile([128, 128], bf16, tag="tp")
                    nc.tensor.transpose(tp[:], ht[:, k * 128:(k + 1) * 128],
                                        ws["ident"].ap())
                    dstT = h2T0 if k == 0 else h2T1
                    nc.scalar.copy(dstT.ap()[:, cols], tp[:])

        scope_l2d.__exit__(None, None, None)
        scope_x2 = nc.named_scope("xchg2"); scope_x2.__enter__()
        # ================= exchange h2 =================
        nc.gpsimd.collective_compute(
            "AllGather", ALU.bypass, replica_groups=[list(range(c["NC"]))],
            ins=[h2_mine.ap().opt()], outs=[h2_full.ap().opt()])

        scope_x2.__exit__(None, None, None)
        if dump:
            nc.sync.dma_start(dbg_h1.ap(), h1_full.ap())
            nc.sync.dma_start(dbg_h2.ap(), h2_full.ap())
            with tc.tile_pool(name="dbg1", bufs=1) as dbgp:
                d1 = dbgp.tile([3, NPC_PAD], f32, tag="d1")
                nc.vector.tensor_copy(d1[:], stack3.ap())
                nc.sync.dma_start(dbg_agg1.ap(), d1[:])

        scope_l3a = nc.named_scope("l3agg"); scope_l3a.__enter__()
        # ---- layer 3 agg: psum [128dst, 256 din] -> agg3 node-major bf16
        def l3_write(t, pt):
            nc.scalar.copy(agg3[:, t, :], pt[:])
        agg_layer(h2_full, 256, l3_write, xg_stationary=False)

        scope_l3a.__exit__(None, None, None)
        scope_l3d = nc.named_scope("l3dense"); scope_l3d.__enter__()
        # ================= LAYER 3 dense + pool =================
        with tc.tile_pool(name="d3p", bufs=3, space="PSUM") as dpsum, \
             tc.tile_pool(name="d3s", bufs=4) as dsb, \
             tc.tile_pool(name="t3p", bufs=3, space="PSUM") as tpsum, \
             tc.tile_pool(name="t3s", bufs=4) as tsb, \
             tc.tile_pool(name="plp", bufs=2, space="PSUM") as plp:
            for t in range(TPC):
                cols = slice(t * 128, (t + 1) * 128)
                a3T = []
                for k in range(2):
                    tp = tpsum.tile([128, 128], bf16, tag="tp")
                    nc.tensor.transpose(tp[:], agg3[:, t, k * 128:(k + 1) * 128],
                                        ws["ident"].ap())
                    sb = tsb.tile([128, 128], bf16, tag="a3T")
                    nc.scalar.copy(sb[:], tp[:])
                    a3T.append(sb)
                zt = dpsum.tile([128, 512], f32, tag="z")
                nc.tensor.matmul(zt[:], a3T[0][:], wsb["wrel3_0"].ap(), start=True, stop=False)
                nc.tensor.matmul(zt[:], a3T[1][:], wsb["wrel3_1"].ap(), start=False, stop=False)
                nc.tensor.matmul(zt[:], h2T0.ap()[:, cols], wsb["wroot3_0"].ap(), start=False, stop=False)
                nc.tensor.matmul(zt[:], h2T1.ap()[:, cols], wsb["wroot3_1"].ap(), start=False, stop=False)
                nc.tensor.matmul(zt[:], ws["onesb"].ap(), wsb["b3"].ap(), start=False, stop=True)
                ht = dsb.tile([128, 512], bf16, tag="h")
                nc.scalar.copy(ht[:], zt[:])
                # pool: B [128, G] one-hot of batch id
                B = dsb.tile([128, G], bf16, tag="B")
                nc.vector.tensor_scalar(B[:], ws["iotaGb"].ap(),
                                        ws["bslot"].ap()[:, t:t + 1], None,
                                        ALU.is_equal)
                pp = plp.tile([G, 512], f32, tag="pp")
                nc.tensor.matmul(pp[:], B[:], ht[:], start=True, stop=True)
                if t == 0:
                    nc.vector.tensor_copy(pooled_acc.ap(), pp[:])
                else:
                    nc.vector.tensor_tensor(pooled_acc.ap(), pooled_acc.ap(),
                                            pp[:], ALU.add)

        scope_l3d.__exit__(None, None, None)
        scope_fin = nc.named_scope("final"); scope_fin.__enter__()
        # ================= allreduce + normalize =================
        if dump:
            nc.sync.dma_start(dbg_pool.ap(), pooled_acc.ap())
        nc.sync.dma_start(pool_in.ap(), pooled_acc.ap())
        nc.gpsimd.collective_compute(
            "AllGather", ALU.bypass, replica_groups=[list(range(c["NC"]))],
            ins=[pool_in.ap().opt()], outs=[pool_out8.ap().opt()])
        with tc.tile_pool(name="fin", bufs=1) as fin:
            ps = fin.tile([G, 512], f32, tag="ps")
            ps8 = fin.tile([G, 8, 512], f32, tag="ps8")
            nc.sync.dma_start(
                ps8[:], pool_out8.ap().rearrange("(r g) f -> g r f", r=8))
            nc.vector.tensor_reduce(ps[:], ps8[:].rearrange("g r f -> g f r"),
                                    mybir.AxisListType.X, ALU.add)
            mean = fin.tile([G, 512], f32, tag="mean")
            nc.vector.tensor_scalar(mean[:], ps[:], ws["invcnt"].ap(), None,
                                    ALU.mult)
            sq = fin.tile([G, 512], f32, tag="sq")
            nc.vector.tensor_tensor(sq[:], mean[:], mean[:], ALU.mult)
            ss = fin.tile([G, 1], f32, tag="ss")
            nc.vector.tensor_reduce(ss[:], sq[:], mybir.AxisListType.X, ALU.add)
            nrm = fin.tile([G, 1], f32, tag="nrm")
            nc.scalar.sqrt(nrm[:], ss[:])
            nc.vector.tensor_scalar(nrm[:], nrm[:], 1e-12, None, ALU.max)
            inv = fin.tile([G, 1], f32, tag="inv")
            nc.vector.reciprocal(inv[:], nrm[:])
            outv = fin.tile([G, 512], f32, tag="outv")
            nc.vector.tensor_scalar(outv[:], mean[:], inv[:], None, ALU.mult)
            nc.sync.dma_start(out_ext.ap(), outv[:])

    scope_fin.__exit__(None, None, None)
    nc.compile()
    return nc


# ----------------------------------------------------------------------------
# In-map assembly
# ----------------------------------------------------------------------------

def make_in_maps(host, inputs, cfg):
    c = cfg
    NC = c["NC"]
    w1stack = np.concatenate([
        np.asarray(inputs["W_rel1"], np.float32).reshape(1, 128),
        np.asarray(inputs["W_root1"], np.float32).reshape(1, 128),
        np.asarray(inputs["b_rel1"], np.float32).reshape(1, 128)], axis=0).astype(BF16)
    shared = {
        "xw": host["xw"],
        "onesrow": host["onesrow"],
        "w1stack": w1stack,
        "wrel2": np.asarray(inputs["W_rel2"], np.float32),
        "wroot2": np.asarray(inputs["W_root2"], np.float32),
        "b2": np.asarray(inputs["b_rel2"], np.float32).reshape(1, 256),
        "wrel3": np.asarray(inputs["W_rel3"], np.float32),
        "wroot3": np.asarray(inputs["W_root3"], np.float32),
        "b3": np.asarray(inputs["b_rel3"], np.float32).reshape(1, 512),
        "ident": host["ident"],
        "iota128": host["iota128"],
        "iota64f": host["iota64f"],
        "iotaGb": host["iotaGb"],
        "onesb": host["onesb"],
        "invcnt": host["invcnt"],
    }
    in_maps = []
    for ci in range(NC):
        m = dict(shared)
        m["xloc"] = host["xloc"][ci]
        m["bslot"] = host["bslot"][ci]
        m["widx"] = host["widx"][ci]
        m["woff"] = host["woff"][ci]
        m["slot1"] = host["slot1"][ci]
        m["gidx"] = host["gidx"][ci]
        m["slotG"] = host["slotG"][ci]
        in_maps.append(m)
    return in_maps


# ----------------------------------------------------------------------------
# Entry points
# ----------------------------------------------------------------------------

_BUILD_CACHE = {}


def _install_ntff_shim(so_path="/opt/axon/libaxon_pjrt.so"):
    """Provide antenv.axon_hooks (absent in this image) so that
    run_bass_kernel_spmd(trace=True) can capture NTFF profiles via the
    axon PJRT plugin's C ABI."""
    import types
    import ctypes
    import contextlib

    if "antenv.axon_hooks" in sys.modules:
        return
    try:
        lib = ctypes.CDLL(so_path)
    except OSError:
        return
    if not hasattr(lib, "axon_start_nrt_profile"):
        return
    lib.axon_start_nrt_profile.argtypes = [
        ctypes.POINTER(ctypes.c_int64), ctypes.c_size_t]
    lib.axon_start_nrt_profile.restype = ctypes.c_int64
    lib.axon_stop_nrt_profile.argtypes = [ctypes.c_char_p]
    lib.axon_stop_nrt_profile.restype = ctypes.c_int64

    @contextlib.contextmanager
    def _hook(output_dir, device_ids):
        import jax
        jax.devices()
        if device_ids:
            ids = (ctypes.c_int64 * len(device_ids))(*device_ids)
            rc = lib.axon_start_nrt_profile(ids, len(device_ids))
        else:
            rc = lib.axon_start_nrt_profile(None, 0)
        if rc != 0:
            raise RuntimeError(f"axon_start_nrt_profile rc={rc}")
        try:
            yield
        finally:
            n = lib.axon_stop_nrt_profile(str(output_dir).encode())
            if n < 0:
                raise RuntimeError(f"axon_stop_nrt_profile rc={n}")
            print(f"profile: {n} file(s) written to {output_dir}")

    mod = types.ModuleType("antenv.axon_hooks")
    mod.get_axon_ntff_profile_hook = lambda: _hook
    mod.set_axon_ntff_profile_hook = lambda h: None
    sys.modules["antenv.axon_hooks"] = mod


def run(inputs, cfg=None, sim=False, trace=False, dump=False):
    cfg = derive(cfg or FULL_CFG)
    host, sched = preprocess(inputs["x"], inputs["edge_index"], inputs["batch"], cfg)
    nc = build_graph(cfg, sched, debug=sim, dump=dump)
    in_maps = make_in_maps(host, inputs, cfg)

    if sim:
        from concourse.bass_interp import MultiCoreSim
        s = MultiCoreSim(nc, num_cores=cfg["NC"])
        for ci in range(cfg["NC"]):
            for k, v in in_maps[ci].items():
                s.cores[ci].tensor(k)[:] = np.ascontiguousarray(v)
        s.simulate(check_with_hw=False)
        out = np.array(s.cores[0].mem_tensor("out"))
        return out, None
    else:
        if trace:
            _install_ntff_shim()
        from concourse import bass_utils
        res = bass_utils.run_bass_kernel_spmd(
            nc, in_maps, core_ids=list(range(cfg["NC"])), trace=trace)
        return np.asarray(res.results[0]["out"]), res


def kernel(**inputs) -> np.ndarray:
    out, _ = run(inputs, FULL_CFG, sim=False, trace=False)
    return out.astype(np.float32)



# revision 8
# speedup vs baseline: 1.0088x; 1.0088x over previous
"""Distributed Trainium2 kernel for 3-layer GraphConv GNN + global mean pool + L2 normalize.

Strategy (8 NeuronCores, SPMD):
  - Nodes sharded by contiguous ranges across cores (dst-sharding of edges).
  - Aggregation (segment_sum of gathered neighbor features) per core:
      * dma_gather pulls h[src] rows from a replicated node-feature table in HBM
        (int16 index limit handled by splitting the table into 32768-row blocks).
      * scatter side is a one-hot matmul into PSUM: for each chunk of <=128 edges,
        PSUM[tile] += onehot(dst_slot)^T-style matmul. Exact f32 accumulation.
  - Dense phase per layer on TensorEngine (bf16 operands, f32 PSUM).
  - h replicated between layers with collective AllGather (bf16).
  - Global mean-pool via batch-one-hot matmul, AllReduce of [G, 512] partials,
    then L2 normalization. All cores produce the full output.

Host-side work is strictly index preprocessing (sorting/partitioning per the
METIS-style sharding hint); no float input values are touched on host.
"""

import math
import sys

import numpy as np

sys.path.insert(0, "/opt/trn_rl_repo")

import ml_dtypes  # noqa: E402

BF16 = ml_dtypes.bfloat16

# ----------------------------------------------------------------------------
# Configs
# ----------------------------------------------------------------------------

FULL_CFG = dict(N=100000, E=800000, G=64, NC=8)
DIMS = [1, 128, 256, 512]
SUPER = 8        # dst tiles per super-iteration (= PSUM banks used)
PADQ = 64        # per-(super,block,tile) segment padding quantum
BLK = 32768      # int16 index block size for gather tables
WIN = 64         # layer-1 f32 gather window (256B)
GCAP = 1024      # max indices per dma_gather call (one ring slot's worth)
SL = 2048        # edges per SBUF slice (gather/one-hot staging, layers 2/3)
SL1 = 2048       # edges per SBUF slice for layer 1
DMA_SCRATCH = 32768  # SWDGE descriptor carveout: 2048 descs/queue = 2 calls in flight


def derive(cfg):
    d = dict(cfg)
    N, NC = d["N"], d["NC"]
    assert N % NC == 0
    d["NPC"] = N // NC
    d["TPC"] = (d["NPC"] + 127) // 128          # node tiles per core
    d["NPC_PAD"] = d["TPC"] * 128
    d["NFULL"] = NC * d["NPC_PAD"]
    d["NBLK"] = (d["NFULL"] + BLK - 1) // BLK
    d["NSUP"] = (d["TPC"] + SUPER - 1) // SUPER
    d["NW1"] = (d["N"] + WIN - 1) // WIN        # x windows
    return d


# ----------------------------------------------------------------------------
# Host preprocessing: edge layout + schedule
# ----------------------------------------------------------------------------

def preprocess(x, edge_index, batch, cfg):
    """Build all per-core host arrays and the static schedule."""
    c = cfg
    N, E, G, NC = c["N"], c["E"], c["G"], c["NC"]
    NPC, TPC, NPC_PAD, NFULL, NBLK, NSUP = (
        c["NPC"], c["TPC"], c["NPC_PAD"], c["NFULL"], c["NBLK"], c["NSUP"])

    src = np.asarray(edge_index[0], dtype=np.int64)
    dst = np.asarray(edge_index[1], dtype=np.int64)
    batch = np.asarray(batch, dtype=np.int64)

    # ---- per-core edge sets
    core_of = dst // NPC
    per_core = []
    for ci in range(NC):
        m = core_of == ci
        es, ed = src[m], dst[m] - ci * NPC
        per_core.append((es, ed))

    # ---- G: main gather layout (shared by layers 2 and 3)
    # order: (super, block, tile, src)
    def g_keys(es, ed):
        tile = ed >> 7
        slot = ed & 127
        sup = tile // SUPER
        spad = (es // NPC) * NPC_PAD + (es % NPC)
        blk = spad // BLK
        return sup, blk, tile, slot, spad

    # segment counts n[core, sup, blk, tile]
    nseg = np.zeros((NC, NSUP, NBLK, TPC), dtype=np.int64)
    gdata = []
    for ci in range(NC):
        es, ed = per_core[ci]
        sup, blk, tile, slot, spad = g_keys(es, ed)
        order = np.lexsort((spad, tile, blk, sup))
        sup, blk, tile, slot, spad = (a[order] for a in (sup, blk, tile, slot, spad))
        np.add.at(nseg[ci], (sup, blk, tile), 1)
        gdata.append((sup, blk, tile, slot, spad))

    nmax = nseg.max(axis=0)  # [NSUP, NBLK, TPC]
    npad = ((nmax + PADQ - 1) // PADQ) * PADQ
    # ensure every (sup, tile) has at least one segment so PSUM gets written
    tile_tot = npad.sum(axis=1)  # [NSUP, TPC]
    for s in range(NSUP):
        for t in range(min(TPC - s * SUPER, SUPER)):
            ti = s * SUPER + t
            if ti < TPC and tile_tot[s, ti] == 0:
                npad[s, 0, ti] = PADQ

    # run = (sup, blk). run length padded to 128.
    run_len = {}
    run_off = {}   # global edge offset of run start
    seg_off = {}   # (s,b,t) -> global offset
    LT = 0
    for s in range(NSUP):
        for b in range(NBLK):
            r0 = LT
            for t in range(TPC):
                if npad[s, b, t]:
                    seg_off[(s, b, t)] = LT
                    LT += int(npad[s, b, t])
            L = LT - r0
            Lp = ((L + 127) // 128) * 128
            LT = r0 + Lp
            run_len[(s, b)] = Lp
            run_off[(s, b)] = r0
    LTG = LT

    # per-core arrays: gidx int16 (block-local padded src), slotG bf16
    gidx = np.zeros((NC, LTG), dtype=np.int16)
    slotG = np.full((NC, LTG), -1.0, dtype=np.float32)
    for ci in range(NC):
        sup, blk, tile, slot, spad = gdata[ci]
        # fill per segment
        pos = 0
        # edges are sorted by (sup, blk, tile); walk segments
        seg_ids = sup * (NBLK * TPC) + blk * TPC + tile
        bounds = np.flatnonzero(np.diff(seg_ids)) + 1
        starts = np.concatenate(([0], bounds))
        ends = np.concatenate((bounds, [len(seg_ids)]))
        for st, en in zip(starts, ends):
            s, b, t = int(sup[st]), int(blk[st]), int(tile[st])
            o = seg_off[(s, b, t)]
            n = en - st
            assert n <= npad[s, b, t]
            loc = spad[st:en] - b * BLK
            assert (loc >= 0).all() and (loc < BLK).all()
            gidx[ci, o:o + n] = loc.astype(np.int16)
            slotG[ci, o:o + n] = slot[st:en].astype(np.float32)
            # pad entries within segment: repeat first idx (slot stays -1)
            gidx[ci, o + n: o + int(npad[s, b, t])] = loc[0] if n else 0
        del pos

    # slice-centric schedule: per s: tiles + runs; each run split into slices
    # of <= SL edges; pieces attached to the slice containing their column.
    SLC = SL // 128
    sched_g = []
    for s in range(NSUP):
        tiles = list(range(s * SUPER, min((s + 1) * SUPER, TPC)))
        # pieces per tile in edge order, with start/stop flags
        runs = []
        for b in range(NBLK):
            L = run_len[(s, b)]
            if not L:
                continue
            ncols = L // 128
            slices = []
            for c0 in range(0, ncols, SLC):
                nc_ = min(SLC, ncols - c0)
                slices.append(dict(c0=c0, ncols=nc_,
                                   off=run_off[(s, b)] + c0 * 128,
                                   num=nc_ * 128, pieces=[]))
            runs.append(dict(b=b, off=run_off[(s, b)], num=L, slices=slices))
        run_by_b = {r["b"]: r for r in runs}
        for t in tiles:
            pieces = []
            for b in range(NBLK):
                if (s, b, t) not in seg_off:
                    continue
                o = seg_off[(s, b, t)]
                ln_tot = int(npad[s, b, t])
                lo = o - run_off[(s, b)]
                while ln_tot > 0:
                    p0 = lo % 128
                    cap = 128 if p0 == 0 else (64 if p0 == 64 else 32)
                    l = min(ln_tot, cap)
                    pieces.append((b, lo // 128, p0, l))
                    lo += l
                    ln_tot -= l
            assert pieces
            for i, (b, col, p0, l) in enumerate(pieces):
                sl = run_by_b[b]["slices"][col // SLC]
                sl["pieces"].append(dict(
                    t=t, col=col - sl["c0"], p0=p0, ln=l,
                    start=(i == 0), stop=(i == len(pieces) - 1)))
        sched_g.append(dict(tiles=tiles, runs=runs))

    # ---- W: layer-1 gather layout: order (super, tile, src)
    def w_keys(es, ed):
        tile = ed >> 7
        slot = ed & 127
        sup = tile // SUPER
        win = es // WIN
        off = es % WIN
        return sup, tile, slot, win, off

    nseg1 = np.zeros((NC, NSUP, TPC), dtype=np.int64)
    wdata = []
    for ci in range(NC):
        es, ed = per_core[ci]
        sup, tile, slot, win, off = w_keys(es, ed)
        order = np.lexsort((win, tile, sup))
        sup, tile, slot, win, off = (a[order] for a in (sup, tile, slot, win, off))
        np.add.at(nseg1[ci], (sup, tile), 1)
        wdata.append((sup, tile, slot, win, off))

    nmax1 = nseg1.max(axis=0)
    npad1 = ((nmax1 + PADQ - 1) // PADQ) * PADQ
    for s in range(NSUP):
        for t in range(s * SUPER, min((s + 1) * SUPER, TPC)):
            if npad1[s, t] == 0:
                npad1[s, t] = PADQ

    seg_off1 = {}
    run_len1 = {}
    run_off1 = {}
    LT = 0
    for s in range(NSUP):
        r0 = LT
        for t in range(TPC):
            if npad1[s, t]:
                seg_off1[(s, t)] = LT
                LT += int(npad1[s, t])
        L = LT - r0
        Lp = ((L + 127) // 128) * 128
        LT = r0 + Lp
        run_len1[s] = Lp
        run_off1[s] = r0
    LT1 = LT

    widx = np.zeros((NC, LT1), dtype=np.int16)
    woff = np.full((NC, LT1), -1.0, dtype=np.float32)
    slot1 = np.full((NC, LT1), -1.0, dtype=np.float32)
    for ci in range(NC):
        sup, tile, slot, win, off = wdata[ci]
        seg_ids = sup * TPC + tile
        bounds = np.flatnonzero(np.diff(seg_ids)) + 1
        starts = np.concatenate(([0], bounds))
        ends = np.concatenate((bounds, [len(seg_ids)]))
        for st, en in zip(starts, ends):
            s, t = int(sup[st]), int(tile[st])
            o = seg_off1[(s, t)]
            n = en - st
            widx[ci, o:o + n] = win[st:en].astype(np.int16)
            woff[ci, o:o + n] = off[st:en].astype(np.float32)
            slot1[ci, o:o + n] = slot[st:en].astype(np.float32)
            widx[ci, o + n:o + int(npad1[s, t])] = win[0] if n else 0

    SLC1 = SL1 // 128
    sched_1 = []
    for s in range(NSUP):
        tiles = list(range(s * SUPER, min((s + 1) * SUPER, TPC)))
        L = run_len1[s]
        ncols = L // 128
        slices = []
        for c0 in range(0, ncols, SLC1):
            nc_ = min(SLC1, ncols - c0)
            slices.append(dict(c0=c0, ncols=nc_, off=run_off1[s] + c0 * 128,
                               num=nc_ * 128, pieces=[]))
        for t in tiles:
            pieces = []
            if (s, t) in seg_off1:
                o = seg_off1[(s, t)]
                ln_tot = int(npad1[s, t])
                lo = o - run_off1[s]
                while ln_tot > 0:
                    p0 = lo % 128
                    cap = 128 if p0 == 0 else (64 if p0 == 64 else 32)
                    l = min(ln_tot, cap)
                    pieces.append((lo // 128, p0, l))
                    lo += l
                    ln_tot -= l
            assert pieces
            for i, (col, p0, l) in enumerate(pieces):
                sl = slices[col // SLC1]
                sl["pieces"].append(dict(
                    t=t, col=col - sl["c0"], p0=p0, ln=l,
                    start=(i == 0), stop=(i == len(pieces) - 1)))
        sched_1.append(dict(tiles=tiles, slices=slices))

    # ---- idx wrap helper: entry i -> [i%16 (+16g), i//16], replicated 8 groups
    def wrap16(a):
        # a: [NC, L] -> [NC, 128, L//16]
        L = a.shape[1]
        assert L % 16 == 0
        w = a.reshape(a.shape[0], L // 16, 16).transpose(0, 2, 1)  # [NC,16,L/16]
        return np.tile(w, (1, 8, 1)).copy()

    def wrap128(a, dtype):
        L = a.shape[1]
        assert L % 128 == 0
        return a.reshape(a.shape[0], L // 128, 128).transpose(0, 2, 1).astype(dtype).copy()

    host = {}
    host["gidx"] = wrap16(gidx)                       # [NC,128,LTG/16] i16
    host["slotG"] = wrap128(slotG, BF16)              # [NC,128,LTG/128]
    host["widx"] = wrap16(widx)                       # [NC,128,LT1/16]
    host["woff"] = wrap128(woff, np.float32)
    host["slot1"] = wrap128(slot1, BF16)

    # ---- x windows, x local, batch slots, counts
    xf = np.asarray(x, dtype=np.float32).reshape(-1)
    xw = np.zeros((c["NW1"] * WIN,), dtype=np.float32)
    xw[:N] = xf
    host["xw"] = xw.reshape(c["NW1"], WIN)

    xloc = np.zeros((NC, 1, NPC_PAD), dtype=np.float32)
    bslot = np.full((NC, NPC_PAD), -1.0, dtype=np.float32)
    for ci in range(NC):
        xloc[ci, 0, :NPC] = xf[ci * NPC:(ci + 1) * NPC]
        bslot[ci, :NPC] = batch[ci * NPC:(ci + 1) * NPC].astype(np.float32)
    host["xloc"] = xloc.astype(BF16)
    # bslot as [128, TPC]: node 128*t+p -> [p, t]
    host["bslot"] = bslot.reshape(NC, TPC, 128).transpose(0, 2, 1).astype(np.float32).copy()

    counts = np.bincount(batch, minlength=G).astype(np.float64)
    host["invcnt"] = (1.0 / np.maximum(counts, 1.0)).astype(np.float32).reshape(G, 1)

    host["onesrow"] = np.ones((1, NPC_PAD), dtype=BF16)
    host["ident"] = np.eye(128, dtype=np.float32).astype(BF16)
    host["iota128"] = np.broadcast_to(
        np.arange(128, dtype=np.float32), (128, 128)).astype(BF16).copy()
    host["iota64f"] = np.broadcast_to(
        np.arange(WIN, dtype=np.float32), (128, WIN)).copy()
    host["iotaGb"] = np.broadcast_to(
        np.arange(G, dtype=np.float32), (128, G)).astype(BF16).copy()
    host["onesb"] = np.ones((1, 128), dtype=np.float32).astype(BF16)

    sched = dict(sched_g=sched_g, sched_1=sched_1, LTG=LTG, LT1=LT1)
    return host, sched


# ----------------------------------------------------------------------------
# Graph builder
# ----------------------------------------------------------------------------

def build_graph(cfg, sched, debug=False, dump=False):
    from concourse import bass, bacc, tile, mybir

    c = cfg
    G = c["G"]
    NPC_PAD, NFULL, TPC, NSUP = c["NPC_PAD"], c["NFULL"], c["TPC"], c["NSUP"]
    f32 = mybir.dt.float32
    bf16 = mybir.dt.bfloat16
    fp8 = mybir.dt.float8e4
    i16 = mybir.dt.int16
    AF = mybir.ActivationFunctionType
    ALU = mybir.AluOpType

    LTG, LT1 = sched["LTG"], sched["LT1"]

    nc = bacc.Bacc("TRN2", target_bir_lowering=False, debug=debug,
                   num_devices=c["NC"], num_swdge_queues=4,
                   dynamic_dma_scratch_size=DMA_SCRATCH)

    # ---------------- dram parameters ----------------
    def din(name, shape, dtype):
        return nc.dram_tensor(name, list(shape), dtype, kind="ExternalInput")

    p = {}
    p["xw"] = din("xw", (c["NW1"], WIN), f32)
    p["xloc"] = din("xloc", (1, NPC_PAD), bf16)
    p["onesrow"] = din("onesrow", (1, NPC_PAD), bf16)
    p["w1stack"] = din("w1stack", (3, 128), bf16)
    p["wrel2"] = din("wrel2", (128, 256), f32)
    p["wroot2"] = din("wroot2", (128, 256), f32)
    p["b2"] = din("b2", (1, 256), f32)
    p["wrel3"] = din("wrel3", (256, 512), f32)
    p["wroot3"] = din("wroot3", (256, 512), f32)
    p["b3"] = din("b3", (1, 512), f32)
    p["ident"] = din("ident", (128, 128), bf16)
    p["iota128"] = din("iota128", (128, 128), bf16)
    p["iota64f"] = din("iota64f", (128, WIN), f32)
    p["iotaGb"] = din("iotaGb", (128, G), bf16)
    p["onesb"] = din("onesb", (1, 128), bf16)
    p["invcnt"] = din("invcnt", (G, 1), f32)
    p["bslot"] = din("bslot", (128, TPC), f32)
    p["widx"] = din("widx", (128, LT1 // 16), i16)
    p["woff"] = din("woff", (128, LT1 // 128), f32)
    p["slot1"] = din("slot1", (128, LT1 // 128), bf16)
    p["gidx"] = din("gidx", (128, LTG // 16), i16)
    p["slotG"] = din("slotG", (128, LTG // 128), bf16)

    out_ext = nc.dram_tensor("out", [G, 512], f32, kind="ExternalOutput")
    if dump:
        dbg_h1 = nc.dram_tensor("dbg_h1", [NFULL, 128], bf16, kind="ExternalOutput")
        dbg_h2 = nc.dram_tensor("dbg_h2", [NFULL, 256], bf16, kind="ExternalOutput")
        dbg_agg1 = nc.dram_tensor("dbg_agg1", [3, NPC_PAD], f32, kind="ExternalOutput")
        dbg_pool = nc.dram_tensor("dbg_pool", [G, 512], f32, kind="ExternalOutput")

    # internal dram
    h1_mine = nc.dram_tensor("h1_mine", [NPC_PAD, 128], bf16)
    h1_full = nc.dram_tensor("h1_full", [NFULL, 128], bf16, addr_space="Shared")
    h2_mine = nc.dram_tensor("h2_mine", [NPC_PAD, 256], bf16)
    h2_full = nc.dram_tensor("h2_full", [NFULL, 256], bf16, addr_space="Shared")
    pool_in = nc.dram_tensor("pool_in", [G, 512], f32)
    pool_out8 = nc.dram_tensor("pool_out8", [8 * G, 512], f32, addr_space="Shared")

    # ---------------- persistent sbuf ----------------
    # arena: h1T / agg2T during L1-L2; agg3 (node-major [128, TPC*256]) in L3
    arena = nc.alloc_sbuf_tensor("arena", [128, 2 * NPC_PAD], bf16)
    h1T = arena.ap()[:, 0:NPC_PAD]
    agg2T = arena.ap()[:, NPC_PAD:2 * NPC_PAD]
    agg3 = arena.ap().rearrange("p (t d) -> p t d", d=256)  # [128, ..., 256]

    h2T0 = nc.alloc_sbuf_tensor("h2T0", [128, NPC_PAD], bf16)
    h2T1 = nc.alloc_sbuf_tensor("h2T1", [128, NPC_PAD], bf16)
    stack3 = nc.alloc_sbuf_tensor("stack3", [3, NPC_PAD], bf16)
    pooled_acc = nc.alloc_sbuf_tensor("pooled_acc", [G, 512], f32)

    ws = {}
    for name, shape, dt_ in [
        ("w1stack", (3, 128), bf16), ("ident", (128, 128), bf16),
        ("iota128", (128, 128), bf16), ("iota64f", (128, WIN), f32),
        ("iotaGb", (128, G), bf16), ("onesb", (1, 128), bf16),
        ("invcnt", (G, 1), f32), ("bslot", (128, TPC), f32),
    ]:
        ws[name] = nc.alloc_sbuf_tensor("sb_" + name, list(shape), dt_)
    # bf16 weights
    wsb = {}
    for name, shape in [("wrel2", (128, 256)), ("wroot2", (128, 256)),
                        ("b2", (1, 256)), ("b3", (1, 512))]:
        wsb[name] = nc.alloc_sbuf_tensor("sbb_" + name, list(shape), bf16)
    for name in ("wrel3", "wroot3"):
        wsb[name + "_0"] = nc.alloc_sbuf_tensor("sbb_" + name + "_0", [128, 512], bf16)
        wsb[name + "_1"] = nc.alloc_sbuf_tensor("sbb_" + name + "_1", [128, 512], bf16)

    with tile.TileContext(nc) as tc:
        # ---------------- load constants ----------------
        with tc.tile_pool(name="wtmp", bufs=2) as wtmp:
            for name in ("w1stack", "ident", "iota128", "iota64f", "iotaGb",
                         "onesb", "invcnt", "bslot"):
                nc.sync.dma_start(ws[name].ap(), p[name].ap())
            for name in ("wrel2", "wroot2", "b2", "b3"):
                t = wtmp.tile(list(p[name].shape), f32, tag="wtmp")
                nc.sync.dma_start(t[:], p[name].ap())
                nc.scalar.copy(wsb[name].ap(), t[:])
            for name in ("wrel3", "wroot3"):
                for k in range(2):
                    t = wtmp.tile([128, 512], f32, tag="wtmp3")
                    nc.sync.dma_start(t[:], p[name].ap()[k * 128:(k + 1) * 128, :])
                    nc.scalar.copy(wsb[name + f"_{k}"].ap(), t[:])
            nc.sync.dma_start(stack3.ap()[1:2, :], p["xloc"].ap())
            nc.sync.dma_start(stack3.ap()[2:3, :], p["onesrow"].ap())

        scope_l1agg = nc.named_scope("l1agg"); scope_l1agg.__enter__()
        # ================= LAYER 1 aggregation =================
        # gather x windows; v = sum(Xg * (iota==off)); psum[1,128] += v^T onehot
        with tc.tile_pool(name="g1", bufs=3) as gpool, \
             tc.tile_pool(name="i1", bufs=4) as ipool, \
             tc.tile_pool(name="s1", bufs=3) as spool, \
             tc.tile_pool(name="p1", bufs=SUPER, space="PSUM") as ppool, \
             tc.tile_pool(name="m1", bufs=3) as mpool:
            for s_ent in sched["sched_1"]:
                pts = {}
                for t in s_ent["tiles"]:
                    pts[t] = ppool.tile([1, 128], f32, tag="ps", name=f"ps1_{t}")
                for sl in s_ent["slices"]:
                    off, num, C = sl["off"], sl["num"], sl["ncols"]
                    idx_t = ipool.tile([128, num // 16], i16, tag="idx")
                    nc.sync.dma_start(idx_t[:], p["widx"].ap()[:, off // 16:(off + num) // 16])
                    xg = gpool.tile([128, C, WIN], f32, tag="g")
                    for e0 in range(0, num, GCAP):
                        n = min(GCAP, num - e0)
                        nc.gpsimd.dma_gather(
                            xg[:, e0 // 128:(e0 + n) // 128, :], p["xw"].ap(),
                            idx_t[:, e0 // 16:(e0 + n) // 16], n, n, WIN,
                            queue_num=(sl["off"] + e0) // GCAP % 4)
                    offc = off // 128
                    woff_t = spool.tile([128, C], f32, tag="woff")
                    nc.scalar.dma_start(woff_t[:], p["woff"].ap()[:, offc:offc + C])
                    slot_t = spool.tile([128, C], bf16, tag="slot")
                    nc.scalar.dma_start(slot_t[:], p["slot1"].ap()[:, offc:offc + C])
                    # mask / v
                    mask = mpool.tile([128, C, WIN], f32, tag="mask")
                    iota_b = ws["iota64f"].ap().rearrange("p w -> p () w").broadcast_to((128, C, WIN))
                    woff_b = woff_t[:].rearrange("p c -> p c ()").broadcast_to((128, C, WIN))
                    nc.vector.tensor_tensor(mask[:], iota_b, woff_b, ALU.is_equal)
                    nc.vector.tensor_tensor(mask[:], mask[:], xg[:], ALU.mult)
                    vf = mpool.tile([128, C], f32, tag="vf")
                    nc.vector.tensor_reduce(vf[:], mask[:], mybir.AxisListType.X, ALU.add)
                    vb = mpool.tile([128, C], bf16, tag="vb")
                    nc.scalar.copy(vb[:], vf[:])
                    # S one-hot
                    S = spool.tile([128, C, 128], bf16, tag="S")
                    iota_s = ws["iota128"].ap().rearrange("p f -> p () f").broadcast_to((128, C, 128))
                    slot_b = slot_t[:].rearrange("p c -> p c ()").broadcast_to((128, C, 128))
                    nc.vector.tensor_tensor(S[:], iota_s, slot_b, ALU.is_equal)
                    for pc in sl["pieces"]:
                        t, col, p0, l = pc["t"], pc["col"], pc["p0"], pc["ln"]
                        nc.tensor.matmul(
                            pts[t][:],
                            vb[p0:p0 + l, col:col + 1],
                            S[p0:p0 + l, col, :],
                            start=pc["start"], stop=pc["stop"])
                for t in s_ent["tiles"]:
                    nc.scalar.copy(stack3.ap()[0:1, t * 128:(t + 1) * 128], pts[t][:])

        scope_l1agg.__exit__(None, None, None)
        scope_l1d = nc.named_scope("l1dense"); scope_l1d.__enter__()
        # ================= LAYER 1 dense =================
        with tc.tile_pool(name="d1p", bufs=3, space="PSUM") as dpsum, \
             tc.tile_pool(name="d1s", bufs=3) as dsb, \
             tc.tile_pool(name="t1p", bufs=2, space="PSUM") as tpsum:
            for t in range(TPC):
                zt = dpsum.tile([128, 128], f32, tag="z")
                nc.tensor.matmul(zt[:], stack3.ap()[:, t * 128:(t + 1) * 128],
                                 ws["w1stack"].ap(), start=True, stop=True)
                ht = dsb.tile([128, 128], bf16, tag="h")
                nc.scalar.activation(ht[:], zt[:], AF.Relu)
                nc.sync.dma_start(h1_mine.ap()[t * 128:(t + 1) * 128, :], ht[:])
                tp = tpsum.tile([128, 128], bf16, tag="tp")
                nc.tensor.transpose(tp[:], ht[:], ws["ident"].ap())
                nc.scalar.copy(h1T[:, t * 128:(t + 1) * 128], tp[:])

        scope_l1d.__exit__(None, None, None)
        scope_x1 = nc.named_scope("xchg1"); scope_x1.__enter__()
        # ================= exchange h1 =================
        nc.gpsimd.collective_compute(
            "AllGather", ALU.bypass, replica_groups=[list(range(c["NC"]))],
            ins=[h1_mine.ap().opt()], outs=[h1_full.ap().opt()])

        # ================= generic aggregation for layers 2/3 =============
        def agg_layer(h_full, d_in, out_write, xg_stationary, xg_dt=bf16):
            """out_write(t, psum_ap): evacuate tile t's psum.

            xg_stationary=True: psum[d_in, 128dst] (lhsT=Xg) — used for L2 so
            the evac lands directly in feature-major agg2T.
            xg_stationary=False: psum[128dst, d_in] (lhsT=S) — used for L3.
            """
            elem = d_in  # bf16 elements per row
            gbufs = 6 if d_in <= 128 else 4
            with tc.tile_pool(name="gA", bufs=gbufs) as gpool, \
                 tc.tile_pool(name="iA", bufs=6) as ipool, \
                 tc.tile_pool(name="sA", bufs=4) as spool, \
                 tc.tile_pool(name="pA", bufs=SUPER, space="PSUM") as ppool:
                shape = [d_in, 128] if xg_stationary else [128, d_in]
                for s_ent in sched["sched_g"]:
                    pts = {}
                    for t in s_ent["tiles"]:
                        pts[t] = ppool.tile(shape, f32, tag="ps", name=f"psA_{t}")
                    for run in s_ent["runs"]:
                        b = run["b"]
                        blk_rows = min(BLK, NFULL - b * BLK)
                        for sl in run["slices"]:
                            off, num, C = sl["off"], sl["num"], sl["ncols"]
                            idx_t = ipool.tile([128, num // 16], i16, tag="idx")
                            nc.sync.dma_start(
                                idx_t[:], p["gidx"].ap()[:, off // 16:(off + num) // 16])
                            xg = gpool.tile([128, C, elem], xg_dt, tag="g")
                            for e0 in range(0, num, GCAP):
                                n = min(GCAP, num - e0)
                                nc.gpsimd.dma_gather(
                                    xg[:, e0 // 128:(e0 + n) // 128, :],
                                    h_full.ap()[b * BLK:b * BLK + blk_rows, :],
                                    idx_t[:, e0 // 16:(e0 + n) // 16], n, n, elem,
                                    queue_num=(sl["off"] + e0) // GCAP % 4)
                            offc = off // 128
                            slot_t = spool.tile([128, C], bf16, tag="slot")
                            nc.scalar.dma_start(slot_t[:], p["slotG"].ap()[:, offc:offc + C])
                            S = spool.tile([128, C, 128], bf16, tag="S")
                            iota_s = ws["iota128"].ap().rearrange("p f -> p () f").broadcast_to((128, C, 128))
                            slot_b = slot_t[:].rearrange("p c -> p c ()").broadcast_to((128, C, 128))
                            nc.vector.tensor_tensor(S[:], iota_s, slot_b, ALU.is_equal)
                            for pc in sl["pieces"]:
                                t, col, p0, l = pc["t"], pc["col"], pc["p0"], pc["ln"]
                                if xg_stationary:
                                    lhsT, rhs = xg[p0:p0 + l, col, :], S[p0:p0 + l, col, :]
                                else:
                                    lhsT, rhs = S[p0:p0 + l, col, :], xg[p0:p0 + l, col, :]
                                nc.tensor.matmul(
                                    pts[t][:], lhsT, rhs,
                                    start=pc["start"], stop=pc["stop"])
                    for t in s_ent["tiles"]:
                        out_write(t, pts[t])

        scope_x1.__exit__(None, None, None)
        scope_l2a = nc.named_scope("l2agg"); scope_l2a.__enter__()
        # ---- layer 2: psum [128din, 128dst] -> agg2T directly
        def l2_write(t, pt):
            nc.scalar.copy(agg2T[:, t * 128:(t + 1) * 128], pt[:])
        agg_layer(h1_full, 128, l2_write, xg_stationary=True)

        scope_l2a.__exit__(None, None, None)
        scope_l2d = nc.named_scope("l2dense"); scope_l2d.__enter__()
        # ================= LAYER 2 dense =================
        with tc.tile_pool(name="d2p", bufs=3, space="PSUM") as dpsum, \
             tc.tile_pool(name="d2s", bufs=4) as dsb, \
             tc.tile_pool(name="t2pp", bufs=3, space="PSUM") as tpsum:
            for t in range(TPC):
                cols = slice(t * 128, (t + 1) * 128)
                zt = dpsum.tile([128, 256], f32, tag="z")
                nc.tensor.matmul(zt[:], agg2T[:, cols], wsb["wrel2"].ap(), start=True, stop=False)
                nc.tensor.matmul(zt[:], h1T[:, cols], wsb["wroot2"].ap(), start=False, stop=False)
                nc.tensor.matmul(zt[:], ws["onesb"].ap(), wsb["b2"].ap(), start=False, stop=True)
                ht = dsb.tile([128, 256], bf16, tag="h")
                nc.scalar.activation(ht[:], zt[:], AF.Relu)
                nc.sync.dma_start(h2_mine.ap()[t * 128:(t + 1) * 128, :], ht[:])
                for k in range(2):
                    tp = tpsum.tile([128, 128], bf16, tag="tp")
                    nc.tensor.transpose(tp[:], ht[:, k * 128:(k + 1) * 128],
                                        ws["ident"].ap())
                    dstT = h2T0 if k == 0 else h2T1
                    nc.scalar.copy(dstT.ap()[:, cols], tp[:])

        scope_l2d.__exit__(None, None, None)
        scope_x2 = nc.named_scope("xchg2"); scope_x2.__enter__()
        # ================= exchange h2 =================
        nc.gpsimd.collective_compute(
            "AllGather", ALU.bypass, replica_groups=[list(range(c["NC"]))],
            ins=[h2_mine.ap().opt()], outs=[h2_full.ap().opt()])

        scope_x2.__exit__(None, None, None)
        if dump:
            nc.sync.dma_start(dbg_h1.ap(), h1_full.ap())
            nc.sync.dma_start(dbg_h2.ap(), h2_full.ap())
            with tc.tile_pool(name="dbg1", bufs=1) as dbgp:
                d1 = dbgp.tile([3, NPC_PAD], f32, tag="d1")
                nc.vector.tensor_copy(d1[:], stack3.ap())
                nc.sync.dma_start(dbg_agg1.ap(), d1[:])

        scope_l3a = nc.named_scope("l3agg"); scope_l3a.__enter__()
        # ---- layer 3 agg: psum [128dst, 256 din] -> agg3 node-major bf16
        def l3_write(t, pt):
            nc.scalar.copy(agg3[:, t, :], pt[:])
        agg_layer(h2_full, 256, l3_write, xg_stationary=False)

        scope_l3a.__exit__(None, None, None)
        scope_l3d = nc.named_scope("l3dense"); scope_l3d.__enter__()
        # ================= LAYER 3 dense + pool =================
        with tc.tile_pool(name="d3p", bufs=3, space="PSUM") as dpsum, \
             tc.tile_pool(name="d3s", bufs=4) as dsb, \
             tc.tile_pool(name="t3p", bufs=3, space="PSUM") as tpsum, \
             tc.tile_pool(name="t3s", bufs=4) as tsb, \
             tc.tile_pool(name="plp", bufs=2, space="PSUM") as plp:
            for t in range(TPC):
                cols = slice(t * 128, (t + 1) * 128)
                a3T = []
                for k in range(2):
                    tp = tpsum.tile([128, 128], bf16, tag="tp")
                    nc.tensor.transpose(tp[:], agg3[:, t, k * 128:(k + 1) * 128],
                                        ws["ident"].ap())
                    sb = tsb.tile([128, 128], bf16, tag="a3T")
                    nc.scalar.copy(sb[:], tp[:])
                    a3T.append(sb)
                zt = dpsum.tile([128, 512], f32, tag="z")
                nc.tensor.matmul(zt[:], a3T[0][:], wsb["wrel3_0"].ap(), start=True, stop=False)
                nc.tensor.matmul(zt[:], a3T[1][:], wsb["wrel3_1"].ap(), start=False, stop=False)
                nc.tensor.matmul(zt[:], h2T0.ap()[:, cols], wsb["wroot3_0"].ap(), start=False, stop=False)
                nc.tensor.matmul(zt[:], h2T1.ap()[:, cols], wsb["wroot3_1"].ap(), start=False, stop=False)
                nc.tensor.matmul(zt[:], ws["onesb"].ap(), wsb["b3"].ap(), start=False, stop=True)
                ht = dsb.tile([128, 512], bf16, tag="h")
                nc.scalar.copy(ht[:], zt[:])
                # pool: B [128, G] one-hot of batch id
                B = dsb.tile([128, G], bf16, tag="B")
                nc.vector.tensor_scalar(B[:], ws["iotaGb"].ap(),
                                        ws["bslot"].ap()[:, t:t + 1], None,
                                        ALU.is_equal)
                pp = plp.tile([G, 512], f32, tag="pp")
                nc.tensor.matmul(pp[:], B[:], ht[:], start=True, stop=True)
                if t == 0:
                    nc.vector.tensor_copy(pooled_acc.ap(), pp[:])
                else:
                    nc.vector.tensor_tensor(pooled_acc.ap(), pooled_acc.ap(),
                                            pp[:], ALU.add)

        scope_l3d.__exit__(None, None, None)
        scope_fin = nc.named_scope("final"); scope_fin.__enter__()
        # ================= allreduce + normalize =================
        if dump:
            nc.sync.dma_start(dbg_pool.ap(), pooled_acc.ap())
        nc.sync.dma_start(pool_in.ap(), pooled_acc.ap())
        nc.gpsimd.collective_compute(
            "AllGather", ALU.bypass, replica_groups=[list(range(c["NC"]))],
            ins=[pool_in.ap().opt()], outs=[pool_out8.ap().opt()])
        with tc.tile_pool(name="fin", bufs=1) as fin:
            ps = fin.tile([G, 512], f32, tag="ps")
            ps8 = fin.tile([G, 8, 512], f32, tag="ps8")
            nc.sync.dma_start(
                ps8[:], pool_out8.ap().rearrange("(r g) f -> g r f", r=8))
            nc.vector.tensor_reduce(ps[:], ps8[:].rearrange("g r f -> g f r"),
                                    mybir.AxisListType.X, ALU.add)
            mean = fin.tile([G, 512], f32, tag="mean")
            nc.vector.tensor_scalar(mean[:], ps[:], ws["invcnt"].ap(), None,
                                    ALU.mult)
            sq = fin.tile([G, 512], f32, tag="sq")
            nc.vector.tensor_tensor(sq[:], mean[:], mean[:], ALU.mult)
            ss = fin.tile([G, 1], f32, tag="ss")
            nc.vector.tensor_reduce(ss[:], sq[:], mybir.AxisListType.X, ALU.add)
            nrm = fin.tile([G, 1], f32, tag="nrm")
            nc.scalar.sqrt(nrm[:], ss[:])
            nc.vector.tensor_scalar(nrm[:], nrm[:], 1e-12, None, ALU.max)
            inv = fin.tile([G, 1], f32, tag="inv")
            nc.vector.reciprocal(inv[:], nrm[:])
            outv = fin.tile([G, 512], f32, tag="outv")
            nc.vector.tensor_scalar(outv[:], mean[:], inv[:], None, ALU.mult)
            nc.sync.dma_start(out_ext.ap(), outv[:])

    scope_fin.__exit__(None, None, None)
    nc.compile()
    return nc


# ----------------------------------------------------------------------------
# In-map assembly
# ----------------------------------------------------------------------------

def make_in_maps(host, inputs, cfg):
    c = cfg
    NC = c["NC"]
    w1stack = np.concatenate([
        np.asarray(inputs["W_rel1"], np.float32).reshape(1, 128),
        np.asarray(inputs["W_root1"], np.float32).reshape(1, 128),
        np.asarray(inputs["b_rel1"], np.float32).reshape(1, 128)], axis=0).astype(BF16)
    shared = {
        "xw": host["xw"],
        "onesrow": host["onesrow"],
        "w1stack": w1stack,
        "wrel2": np.asarray(inputs["W_rel2"], np.float32),
        "wroot2": np.asarray(inputs["W_root2"], np.float32),
        "b2": np.asarray(inputs["b_rel2"], np.float32).reshape(1, 256),
        "wrel3": np.asarray(inputs["W_rel3"], np.float32),
        "wroot3": np.asarray(inputs["W_root3"], np.float32),
        "b3": np.asarray(inputs["b_rel3"], np.float32).reshape(1, 512),
        "ident": host["ident"],
        "iota128": host["iota128"],
        "iota64f": host["iota64f"],
        "iotaGb": host["iotaGb"],
        "onesb": host["onesb"],
        "invcnt": host["invcnt"],
    }
    in_maps = []
    for ci in range(NC):
        m = dict(shared)
        m["xloc"] = host["xloc"][ci]
        m["bslot"] = host["bslot"][ci]
        m["widx"] = host["widx"][ci]
        m["woff"] = host["woff"][ci]
        m["slot1"] = host["slot1"][ci]
        m["gidx"] = host["gidx"][ci]
        m["slotG"] = host["slotG"][ci]
        in_maps.append(m)
    return in_maps


# ----------------------------------------------------------------------------
# Entry points
# ----------------------------------------------------------------------------

_BUILD_CACHE = {}


def _install_ntff_shim(so_path="/opt/axon/libaxon_pjrt.so"):
    """Provide antenv.axon_hooks (absent in this image) so that
    run_bass_kernel_spmd(trace=True) can capture NTFF profiles via the
    axon PJRT plugin's C ABI."""
    import types
    import ctypes
    import contextlib

    if "antenv.axon_hooks" in sys.modules:
        return
    try:
        lib = ctypes.CDLL(so_path)
    except OSError:
        return
    if not hasattr(lib, "axon_start_nrt_profile"):
        return
    lib.axon_start_nrt_profile.argtypes = [
        ctypes.POINTER(ctypes.c_int64), ctypes.c_size_t]
    lib.axon_start_nrt_profile.restype = ctypes.c_int64
    lib.axon_stop_nrt_profile.argtypes = [ctypes.c_char_p]
    lib.axon_stop_nrt_profile.restype = ctypes.c_int64

    @contextlib.contextmanager
    def _hook(output_dir, device_ids):
        import jax
        jax.devices()
        if device_ids:
            ids = (ctypes.c_int64 * len(device_ids))(*device_ids)
            rc = lib.axon_start_nrt_profile(ids, len(device_ids))
        else:
            rc = lib.axon_start_nrt_profile(None, 0)
        if rc != 0:
            raise RuntimeError(f"axon_start_nrt_profile rc={rc}")
        try:
            yield
        finally:
            n = lib.axon_stop_nrt_profile(str(output_dir).encode())
            if n < 0:
                raise RuntimeError(f"axon_stop_nrt_profile rc={n}")
            print(f"profile: {n} file(s) written to {output_dir}")

    mod = types.ModuleType("antenv.axon_hooks")
    mod.get_axon_ntff_profile_hook = lambda: _hook
    mod.set_axon_ntff_profile_hook = lambda h: None
    sys.modules["antenv.axon_hooks"] = mod


def run(inputs, cfg=None, sim=False, trace=False, dump=False):
    cfg = derive(cfg or FULL_CFG)
    host, sched = preprocess(inputs["x"], inputs["edge_index"], inputs["batch"], cfg)
    nc = build_graph(cfg, sched, debug=sim, dump=dump)
    in_maps = make_in_maps(host, inputs, cfg)

    if sim:
        from concourse.bass_interp import MultiCoreSim
        s = MultiCoreSim(nc, num_cores=cfg["NC"])
        for ci in range(cfg["NC"]):
            for k, v in in_maps[ci].items():
                s.cores[ci].tensor(k)[:] = np.ascontiguousarray(v)
        s.simulate(check_with_hw=False)
        out = np.array(s.cores[0].mem_tensor("out"))
        return out, None
    else:
        if trace:
            _install_ntff_shim()
        from concourse import bass_utils
        res = bass_utils.run_bass_kernel_spmd(
            nc, in_maps, core_ids=list(range(cfg["NC"])), trace=trace)
        return np.asarray(res.results[0]["out"]), res


def kernel(**inputs) -> np.ndarray:
    out, _ = run(inputs, FULL_CFG, sim=False, trace=False)
    return out.astype(np.float32)



# revision 15
# speedup vs baseline: 1.1778x; 1.1676x over previous
"""Distributed Trainium2 kernel for 3-layer GraphConv GNN + global mean pool + L2 normalize.

Strategy (8 NeuronCores, SPMD):
  - Nodes sharded by contiguous ranges across cores (dst-sharding of edges).
  - Aggregation (segment_sum of gathered neighbor features) per core:
      * dma_gather pulls h[src] rows from a replicated node-feature table in HBM
        (int16 index limit handled by splitting the table into 32768-row blocks).
      * scatter side is a one-hot matmul into PSUM: for each chunk of <=128 edges,
        PSUM[tile] += onehot(dst_slot)^T-style matmul. Exact f32 accumulation.
  - Dense phase per layer on TensorEngine (bf16 operands, f32 PSUM).
  - h replicated between layers with collective AllGather (bf16).
  - Global mean-pool via batch-one-hot matmul, AllReduce of [G, 512] partials,
    then L2 normalization. All cores produce the full output.

Host-side work is strictly index preprocessing (sorting/partitioning per the
METIS-style sharding hint); no float input values are touched on host.
"""

import math
import sys

import numpy as np

sys.path.insert(0, "/opt/trn_rl_repo")

import ml_dtypes  # noqa: E402

BF16 = ml_dtypes.bfloat16

# ----------------------------------------------------------------------------
# Configs
# ----------------------------------------------------------------------------

FULL_CFG = dict(N=100000, E=800000, G=64, NC=8)
DIMS = [1, 128, 256, 512]
SUPERG = 5       # dst tiles per super-iteration, layers 2/3 (PSUM banks: 5+3 dense)
SUPER1 = 6       # dst tiles per super-iteration, layer 1 (PSUM banks: 6+2 dense)
PADQ = 64        # per-(super,quarter,tile) segment padding quantum
NQ = 4           # node-table quarters (pipelined AllGather granularity)
WIN = 64         # layer-1 f32 gather window (256B)
GCAP = 1024      # max indices per dma_gather call (one ring slot's worth)
SL = 2048        # edges per SBUF slice (gather/one-hot staging, layers 2/3)
SL1 = 2048       # edges per SBUF slice for layer 1
DMA_SCRATCH = 32768  # SWDGE descriptor carveout: 2048 descs/queue = 2 calls in flight


def derive(cfg):
    d = dict(cfg)
    N, NC = d["N"], d["NC"]
    assert N % NC == 0
    d["NPC"] = N // NC
    d["TPC"] = (d["NPC"] + 127) // 128          # node tiles per core
    d["NPC_PAD"] = d["TPC"] * 128
    d["NFULL"] = NC * d["NPC_PAD"]
    # quarter q covers tiles [QT0*q, ...): first NQ-1 quarters equal, last takes rest
    qt = (d["TPC"] + NQ - 1) // NQ
    d["QTILES"] = [qt] * (NQ - 1) + [d["TPC"] - qt * (NQ - 1)]
    d["QROWS"] = [t * 128 for t in d["QTILES"]]
    d["QSTART"] = [qt * 128 * q for q in range(NQ)]
    d["QENDT"] = [qt * (q + 1) for q in range(NQ - 1)] + [d["TPC"]]
    d["NSUP"] = (d["TPC"] + SUPERG - 1) // SUPERG
    d["NSUP1"] = (d["TPC"] + SUPER1 - 1) // SUPER1
    d["NW1"] = (d["N"] + WIN - 1) // WIN        # x windows
    return d


# ----------------------------------------------------------------------------
# Host preprocessing: edge layout + schedule
# ----------------------------------------------------------------------------

def preprocess(x, edge_index, batch, cfg):
    """Build all per-core host arrays and the static schedule."""
    c = cfg
    N, E, G, NC = c["N"], c["E"], c["G"], c["NC"]
    NPC, TPC, NPC_PAD, NFULL, NSUP, NSUP1 = (
        c["NPC"], c["TPC"], c["NPC_PAD"], c["NFULL"], c["NSUP"], c["NSUP1"])
    NBLK = NQ
    QROWS, QSTART = c["QROWS"], c["QSTART"]
    qt0 = c["QTILES"][0]

    src = np.asarray(edge_index[0], dtype=np.int64)
    dst = np.asarray(edge_index[1], dtype=np.int64)
    batch = np.asarray(batch, dtype=np.int64)

    # ---- per-core edge sets
    core_of = dst // NPC
    per_core = []
    for ci in range(NC):
        m = core_of == ci
        es, ed = src[m], dst[m] - ci * NPC
        per_core.append((es, ed))

    qrows_np = np.asarray(QROWS)
    qstart_np = np.asarray(QSTART)

    # ---- G: main gather layout (shared by layers 2 and 3)
    # order: (super, quarter, tile, src). Quarter q's table holds rows
    # [QSTART[q], QSTART[q]+QROWS[q]) of every core's shard, concatenated by
    # rank (the AllGather output layout).
    def g_keys(es, ed):
        tile = ed >> 7
        slot = ed & 127
        sup = tile // SUPERG
        cb = es // NPC
        r = es % NPC
        blk = np.minimum(r >> 7, TPC - 1) // qt0
        spad = cb * qrows_np[blk] + (r - qstart_np[blk])
        return sup, blk, tile, slot, spad

    # segment counts n[core, sup, blk, tile]
    nseg = np.zeros((NC, NSUP, NBLK, TPC), dtype=np.int64)
    gdata = []
    for ci in range(NC):
        es, ed = per_core[ci]
        sup, blk, tile, slot, spad = g_keys(es, ed)
        order = np.lexsort((spad, tile, blk, sup))
        sup, blk, tile, slot, spad = (a[order] for a in (sup, blk, tile, slot, spad))
        np.add.at(nseg[ci], (sup, blk, tile), 1)
        gdata.append((sup, blk, tile, slot, spad))

    nmax = nseg.max(axis=0)  # [NSUP, NBLK, TPC]
    npad = ((nmax + PADQ - 1) // PADQ) * PADQ
    # ensure every (sup, tile) has at least one segment so PSUM gets written
    tile_tot = npad.sum(axis=1)  # [NSUP, TPC]
    for s in range(NSUP):
        for t in range(min(TPC - s * SUPERG, SUPERG)):
            ti = s * SUPERG + t
            if ti < TPC and tile_tot[s, ti] == 0:
                npad[s, 0, ti] = PADQ

    # run = (sup, blk). run length padded to 128.
    run_len = {}
    run_off = {}   # global edge offset of run start
    seg_off = {}   # (s,b,t) -> global offset
    LT = 0
    for s in range(NSUP):
        for b in range(NBLK):
            r0 = LT
            for t in range(TPC):
                if npad[s, b, t]:
                    seg_off[(s, b, t)] = LT
                    LT += int(npad[s, b, t])
            L = LT - r0
            Lp = ((L + 127) // 128) * 128
            LT = r0 + Lp
            run_len[(s, b)] = Lp
            run_off[(s, b)] = r0
    LTG = LT

    # per-core arrays: gidx int16 (block-local padded src), slotG bf16
    gidx = np.zeros((NC, LTG), dtype=np.int16)
    slotG = np.full((NC, LTG), -1.0, dtype=np.float32)
    for ci in range(NC):
        sup, blk, tile, slot, spad = gdata[ci]
        # fill per segment
        pos = 0
        # edges are sorted by (sup, blk, tile); walk segments
        seg_ids = sup * (NBLK * TPC) + blk * TPC + tile
        bounds = np.flatnonzero(np.diff(seg_ids)) + 1
        starts = np.concatenate(([0], bounds))
        ends = np.concatenate((bounds, [len(seg_ids)]))
        for st, en in zip(starts, ends):
            s, b, t = int(sup[st]), int(blk[st]), int(tile[st])
            o = seg_off[(s, b, t)]
            n = en - st
            assert n <= npad[s, b, t]
            loc = spad[st:en]
            assert (loc >= 0).all() and (loc < NC * QROWS[b]).all()
            gidx[ci, o:o + n] = loc.astype(np.int16)
            slotG[ci, o:o + n] = slot[st:en].astype(np.float32)
            # pad entries within segment: repeat first idx (slot stays -1)
            gidx[ci, o + n: o + int(npad[s, b, t])] = loc[0] if n else 0
        del pos

    # slice-centric schedule: per s: tiles + runs; each run split into slices
    # of <= SL edges; pieces attached to the slice containing their column.
    SLC = SL // 128
    sched_g = []
    for s in range(NSUP):
        tiles = list(range(s * SUPERG, min((s + 1) * SUPERG, TPC)))
        # pieces per tile in edge order, with start/stop flags
        runs = []
        for b in range(NBLK):
            L = run_len[(s, b)]
            if not L:
                continue
            ncols = L // 128
            slices = []
            for c0 in range(0, ncols, SLC):
                nc_ = min(SLC, ncols - c0)
                slices.append(dict(c0=c0, ncols=nc_,
                                   off=run_off[(s, b)] + c0 * 128,
                                   num=nc_ * 128, pieces=[]))
            runs.append(dict(b=b, off=run_off[(s, b)], num=L, slices=slices))
        run_by_b = {r["b"]: r for r in runs}
        for t in tiles:
            pieces = []
            for b in range(NBLK):
                if (s, b, t) not in seg_off:
                    continue
                o = seg_off[(s, b, t)]
                ln_tot = int(npad[s, b, t])
                lo = o - run_off[(s, b)]
                while ln_tot > 0:
                    p0 = lo % 128
                    cap = 128 if p0 == 0 else (64 if p0 == 64 else 32)
                    l = min(ln_tot, cap)
                    pieces.append((b, lo // 128, p0, l))
                    lo += l
                    ln_tot -= l
            assert pieces
            for i, (b, col, p0, l) in enumerate(pieces):
                sl = run_by_b[b]["slices"][col // SLC]
                sl["pieces"].append(dict(
                    t=t, col=col - sl["c0"], p0=p0, ln=l,
                    start=(i == 0), stop=(i == len(pieces) - 1)))
        sched_g.append(dict(tiles=tiles, runs=runs))

    # ---- W: layer-1 gather layout: order (super, tile, src)
    def w_keys(es, ed):
        tile = ed >> 7
        slot = ed & 127
        sup = tile // SUPER1
        win = es // WIN
        off = es % WIN
        return sup, tile, slot, win, off

    nseg1 = np.zeros((NC, NSUP1, TPC), dtype=np.int64)
    wdata = []
    for ci in range(NC):
        es, ed = per_core[ci]
        sup, tile, slot, win, off = w_keys(es, ed)
        order = np.lexsort((win, tile, sup))
        sup, tile, slot, win, off = (a[order] for a in (sup, tile, slot, win, off))
        np.add.at(nseg1[ci], (sup, tile), 1)
        wdata.append((sup, tile, slot, win, off))

    nmax1 = nseg1.max(axis=0)
    npad1 = ((nmax1 + PADQ - 1) // PADQ) * PADQ
    for s in range(NSUP1):
        for t in range(s * SUPER1, min((s + 1) * SUPER1, TPC)):
            if npad1[s, t] == 0:
                npad1[s, t] = PADQ

    seg_off1 = {}
    run_len1 = {}
    run_off1 = {}
    LT = 0
    for s in range(NSUP1):
        r0 = LT
        for t in range(TPC):
            if npad1[s, t]:
                seg_off1[(s, t)] = LT
                LT += int(npad1[s, t])
        L = LT - r0
        Lp = ((L + 127) // 128) * 128
        LT = r0 + Lp
        run_len1[s] = Lp
        run_off1[s] = r0
    LT1 = LT

    widx = np.zeros((NC, LT1), dtype=np.int16)
    woff = np.full((NC, LT1), -1.0, dtype=np.float32)
    slot1 = np.full((NC, LT1), -1.0, dtype=np.float32)
    for ci in range(NC):
        sup, tile, slot, win, off = wdata[ci]
        seg_ids = sup * TPC + tile
        bounds = np.flatnonzero(np.diff(seg_ids)) + 1
        starts = np.concatenate(([0], bounds))
        ends = np.concatenate((bounds, [len(seg_ids)]))
        for st, en in zip(starts, ends):
            s, t = int(sup[st]), int(tile[st])
            o = seg_off1[(s, t)]
            n = en - st
            widx[ci, o:o + n] = win[st:en].astype(np.int16)
            woff[ci, o:o + n] = off[st:en].astype(np.float32)
            slot1[ci, o:o + n] = slot[st:en].astype(np.float32)
            widx[ci, o + n:o + int(npad1[s, t])] = win[0] if n else 0

    SLC1 = SL1 // 128
    sched_1 = []
    for s in range(NSUP1):
        tiles = list(range(s * SUPER1, min((s + 1) * SUPER1, TPC)))
        L = run_len1[s]
        ncols = L // 128
        slices = []
        for c0 in range(0, ncols, SLC1):
            nc_ = min(SLC1, ncols - c0)
            slices.append(dict(c0=c0, ncols=nc_, off=run_off1[s] + c0 * 128,
                               num=nc_ * 128, pieces=[]))
        for t in tiles:
            pieces = []
            if (s, t) in seg_off1:
                o = seg_off1[(s, t)]
                ln_tot = int(npad1[s, t])
                lo = o - run_off1[s]
                while ln_tot > 0:
                    p0 = lo % 128
                    cap = 128 if p0 == 0 else (64 if p0 == 64 else 32)
                    l = min(ln_tot, cap)
                    pieces.append((lo // 128, p0, l))
                    lo += l
                    ln_tot -= l
            assert pieces
            for i, (col, p0, l) in enumerate(pieces):
                sl = slices[col // SLC1]
                sl["pieces"].append(dict(
                    t=t, col=col - sl["c0"], p0=p0, ln=l,
                    start=(i == 0), stop=(i == len(pieces) - 1)))
        sched_1.append(dict(tiles=tiles, slices=slices))

    # ---- idx wrap helper: entry i -> [i%16 (+16g), i//16], replicated 8 groups
    def wrap16(a):
        # a: [NC, L] -> [NC, 128, L//16]
        L = a.shape[1]
        assert L % 16 == 0
        w = a.reshape(a.shape[0], L // 16, 16).transpose(0, 2, 1)  # [NC,16,L/16]
        return np.tile(w, (1, 8, 1)).copy()

    def wrap128(a, dtype):
        L = a.shape[1]
        assert L % 128 == 0
        return a.reshape(a.shape[0], L // 128, 128).transpose(0, 2, 1).astype(dtype).copy()

    host = {}
    host["gidx"] = wrap16(gidx)                       # [NC,128,LTG/16] i16
    host["slotG"] = wrap128(slotG, BF16)              # [NC,128,LTG/128]
    host["widx"] = wrap16(widx)                       # [NC,128,LT1/16]
    host["woff"] = wrap128(woff, np.float32)
    host["slot1"] = wrap128(slot1, BF16)

    # ---- x windows, x local, batch slots, counts
    xf = np.asarray(x, dtype=np.float32).reshape(-1)
    xw = np.zeros((c["NW1"] * WIN,), dtype=np.float32)
    xw[:N] = xf
    host["xw"] = xw.reshape(c["NW1"], WIN)

    xloc = np.zeros((NC, 1, NPC_PAD), dtype=np.float32)
    bslot = np.full((NC, NPC_PAD), -1.0, dtype=np.float32)
    for ci in range(NC):
        xloc[ci, 0, :NPC] = xf[ci * NPC:(ci + 1) * NPC]
        bslot[ci, :NPC] = batch[ci * NPC:(ci + 1) * NPC].astype(np.float32)
    host["xloc"] = xloc.astype(BF16)
    # bslot as [128, TPC]: node 128*t+p -> [p, t]
    host["bslot"] = bslot.reshape(NC, TPC, 128).transpose(0, 2, 1).astype(np.float32).copy()

    counts = np.bincount(batch, minlength=G).astype(np.float64)
    host["invcnt"] = (1.0 / np.maximum(counts, 1.0)).astype(np.float32).reshape(G, 1)

    host["onesrow"] = np.ones((1, NPC_PAD), dtype=BF16)
    host["ident"] = np.eye(128, dtype=np.float32).astype(BF16)
    host["iota128"] = np.broadcast_to(
        np.arange(128, dtype=np.float32), (128, 128)).astype(BF16).copy()
    host["iota64f"] = np.broadcast_to(
        np.arange(WIN, dtype=np.float32), (128, WIN)).copy()
    host["iotaGb"] = np.broadcast_to(
        np.arange(G, dtype=np.float32), (128, G)).astype(BF16).copy()
    host["onesb"] = np.ones((1, 128), dtype=np.float32).astype(BF16)

    sched = dict(sched_g=sched_g, sched_1=sched_1, LTG=LTG, LT1=LT1)
    return host, sched


# ----------------------------------------------------------------------------
# Graph builder
# ----------------------------------------------------------------------------

def build_graph(cfg, sched, debug=False, dump=False):
    from concourse import bass, bacc, tile, mybir

    c = cfg
    G = c["G"]
    NC = c["NC"]
    NPC_PAD, NFULL, TPC = c["NPC_PAD"], c["NFULL"], c["TPC"]
    QROWS, QSTART, QENDT = c["QROWS"], c["QSTART"], c["QENDT"]
    f32 = mybir.dt.float32
    bf16 = mybir.dt.bfloat16
    i16 = mybir.dt.int16
    AF = mybir.ActivationFunctionType
    ALU = mybir.AluOpType

    LTG, LT1 = sched["LTG"], sched["LT1"]

    nc = bacc.Bacc("TRN2", target_bir_lowering=False, debug=debug,
                   num_devices=NC, num_swdge_queues=4,
                   dynamic_dma_scratch_size=DMA_SCRATCH)

    # ---------------- dram parameters ----------------
    def din(name, shape, dtype):
        return nc.dram_tensor(name, list(shape), dtype, kind="ExternalInput")

    p = {}
    p["xw"] = din("xw", (c["NW1"], WIN), f32)
    p["xloc"] = din("xloc", (1, NPC_PAD), bf16)
    p["onesrow"] = din("onesrow", (1, NPC_PAD), bf16)
    p["w1stack"] = din("w1stack", (3, 128), bf16)
    p["wrel2"] = din("wrel2", (128, 256), f32)
    p["wroot2"] = din("wroot2", (128, 256), f32)
    p["b2"] = din("b2", (1, 256), f32)
    p["wrel3"] = din("wrel3", (256, 512), f32)
    p["wroot3"] = din("wroot3", (256, 512), f32)
    p["b3"] = din("b3", (1, 512), f32)
    p["ident"] = din("ident", (128, 128), bf16)
    p["iota128"] = din("iota128", (128, 128), bf16)
    p["iota64f"] = din("iota64f", (128, WIN), f32)
    p["iotaGb"] = din("iotaGb", (128, G), bf16)
    p["onesb"] = din("onesb", (1, 128), bf16)
    p["invcnt"] = din("invcnt", (G, 1), f32)
    p["bslot"] = din("bslot", (128, TPC), f32)
    p["widx"] = din("widx", (128, LT1 // 16), i16)
    p["woff"] = din("woff", (128, LT1 // 128), f32)
    p["slot1"] = din("slot1", (128, LT1 // 128), bf16)
    p["gidx"] = din("gidx", (128, LTG // 16), i16)
    p["slotG"] = din("slotG", (128, LTG // 128), bf16)

    out_ext = nc.dram_tensor("out", [G, 512], f32, kind="ExternalOutput")

    # internal dram
    h1_mine = nc.dram_tensor("h1_mine", [NPC_PAD, 128], bf16)
    h2_mine = nc.dram_tensor("h2_mine", [NPC_PAD, 256], bf16)
    h1q = [nc.dram_tensor(f"h1q{q}", [NC * QROWS[q], 128], bf16,
                          addr_space="Shared") for q in range(NQ)]
    h2q = [nc.dram_tensor(f"h2q{q}", [NC * QROWS[q], 256], bf16,
                          addr_space="Shared") for q in range(NQ)]
    pool_in = nc.dram_tensor("pool_in", [G, 512], f32)
    pool_out8 = nc.dram_tensor("pool_out8", [8 * G, 512], f32, addr_space="Shared")

    # ---------------- persistent sbuf ----------------
    # arena: h1T / agg2T during L1-L2; agg3 (node-major [128, TPC*256]) in L3
    arena = nc.alloc_sbuf_tensor("arena", [128, 2 * NPC_PAD], bf16)
    h1T = arena.ap()[:, 0:NPC_PAD]
    agg2T = arena.ap()[:, NPC_PAD:2 * NPC_PAD]
    agg3 = arena.ap().rearrange("p (t d) -> p t d", d=256)  # [128, ..., 256]

    h2T0 = nc.alloc_sbuf_tensor("h2T0", [128, NPC_PAD], bf16)
    h2T1 = nc.alloc_sbuf_tensor("h2T1", [128, NPC_PAD], bf16)
    pooled_acc = nc.alloc_sbuf_tensor("pooled_acc", [G, 512], f32)

    ws = {}
    for name, shape, dt_ in [
        ("w1stack", (3, 128), bf16), ("ident", (128, 128), bf16),
        ("iota128", (128, 128), bf16), ("iota64f", (128, WIN), f32),
        ("iotaGb", (128, G), bf16), ("onesb", (1, 128), bf16),
        ("invcnt", (G, 1), f32), ("bslot", (128, TPC), f32),
    ]:
        ws[name] = nc.alloc_sbuf_tensor("sb_" + name, list(shape), dt_)
    # bf16 weights
    wsb = {}
    for name, shape in [("wrel2", (128, 256)), ("wroot2", (128, 256)),
                        ("b2", (1, 256)), ("b3", (1, 512))]:
        wsb[name] = nc.alloc_sbuf_tensor("sbb_" + name, list(shape), bf16)
    for name in ("wrel3", "wroot3"):
        wsb[name + "_0"] = nc.alloc_sbuf_tensor("sbb_" + name + "_0", [128, 512], bf16)
        wsb[name + "_1"] = nc.alloc_sbuf_tensor("sbb_" + name + "_1", [128, 512], bf16)

    groups = [list(range(NC))]

    with tile.TileContext(nc) as tc:
        # ---------------- load constants ----------------
        with tc.tile_pool(name="wtmp", bufs=2) as wtmp:
            for name in ("w1stack", "ident", "iota128", "iota64f", "iotaGb",
                         "onesb", "invcnt", "bslot"):
                nc.sync.dma_start(ws[name].ap(), p[name].ap())
            for name in ("wrel2", "wroot2", "b2", "b3"):
                t = wtmp.tile(list(p[name].shape), f32, tag="wtmp")
                nc.sync.dma_start(t[:], p[name].ap())
                nc.scalar.copy(wsb[name].ap(), t[:])
            for name in ("wrel3", "wroot3"):
                for k in range(2):
                    t = wtmp.tile([128, 512], f32, tag="wtmp3")
                    nc.sync.dma_start(t[:], p[name].ap()[k * 128:(k + 1) * 128, :])
                    nc.scalar.copy(wsb[name + f"_{k}"].ap(), t[:])

        # ======================================================================
        # LAYER 1: gather-aggregate + interleaved dense + quarter AllGathers
        # ======================================================================
        with tc.tile_pool(name="streams1", bufs=1) as stp1, \
             tc.tile_pool(name="stack3p", bufs=1) as s3p:
            widx_r = stp1.tile([128, LT1 // 16], i16, tag="widx")
            nc.sync.dma_start(widx_r[:], p["widx"].ap())
            woff_r = stp1.tile([128, LT1 // 128], f32, tag="woff")
            nc.sync.dma_start(woff_r[:], p["woff"].ap())
            slot1_r = stp1.tile([128, LT1 // 128], bf16, tag="slot1")
            nc.sync.dma_start(slot1_r[:], p["slot1"].ap())
            stack3 = s3p.tile([3, NPC_PAD], bf16, tag="stack3")
            nc.sync.dma_start(stack3[1:2, :], p["xloc"].ap())
            nc.sync.dma_start(stack3[2:3, :], p["onesrow"].ap())

            scope_l1 = nc.named_scope("l1"); scope_l1.__enter__()
            with tc.tile_pool(name="g1", bufs=3) as gpool, \
                 tc.tile_pool(name="s1", bufs=3) as spool, \
                 tc.tile_pool(name="m1", bufs=3) as mpool, \
                 tc.tile_pool(name="p1", bufs=SUPER1, space="PSUM") as ppool, \
                 tc.tile_pool(name="d1p", bufs=1, space="PSUM") as dpsum, \
                 tc.tile_pool(name="t1p", bufs=1, space="PSUM") as tpsum, \
                 tc.tile_pool(name="d1s", bufs=2) as dsb:
                next_q = [0]
                tiles_done = [0]

                def l1_quarters():
                    while next_q[0] < NQ and tiles_done[0] >= QENDT[next_q[0]]:
                        q = next_q[0]
                        nc.gpsimd.collective_compute(
                            "AllGather", ALU.bypass, replica_groups=groups,
                            ins=[h1_mine.ap()[QSTART[q]:QSTART[q] + QROWS[q], :].opt()],
                            outs=[h1q[q].ap().opt()])
                        next_q[0] += 1

                for s_ent in sched["sched_1"]:
                    pts = {}
                    for t in s_ent["tiles"]:
                        pts[t] = ppool.tile([1, 128], f32, tag="ps", name=f"ps1_{t}")
                    for sl in s_ent["slices"]:
                        off, num, C = sl["off"], sl["num"], sl["ncols"]
                        xg = gpool.tile([128, C, WIN], f32, tag="g")
                        for e0 in range(0, num, GCAP):
                            n = min(GCAP, num - e0)
                            nc.gpsimd.dma_gather(
                                xg[:, e0 // 128:(e0 + n) // 128, :], p["xw"].ap(),
                                widx_r[:, (off + e0) // 16:(off + e0 + n) // 16],
                                n, n, WIN,
                                queue_num=(off + e0) // GCAP % 4)
                        offc = off // 128
                        # mask / v
                        mask = mpool.tile([128, C, WIN], f32, tag="mask")
                        iota_b = ws["iota64f"].ap().rearrange("p w -> p () w").broadcast_to((128, C, WIN))
                        woff_b = woff_r[:, offc:offc + C].rearrange("p c -> p c ()").broadcast_to((128, C, WIN))
                        nc.vector.tensor_tensor(mask[:], iota_b, woff_b, ALU.is_equal)
                        nc.vector.tensor_tensor(mask[:], mask[:], xg[:], ALU.mult)
                        vf = mpool.tile([128, C], f32, tag="vf")
                        nc.vector.tensor_reduce(vf[:], mask[:], mybir.AxisListType.X, ALU.add)
                        vb = mpool.tile([128, C], bf16, tag="vb")
                        nc.scalar.copy(vb[:], vf[:])
                        # S one-hot
                        S = spool.tile([128, C, 128], bf16, tag="S")
                        iota_s = ws["iota128"].ap().rearrange("p f -> p () f").broadcast_to((128, C, 128))
                        slot_b = slot1_r[:, offc:offc + C].rearrange("p c -> p c ()").broadcast_to((128, C, 128))
                        nc.vector.tensor_tensor(S[:], iota_s, slot_b, ALU.is_equal)
                        for pc in sl["pieces"]:
                            t, col, p0, l = pc["t"], pc["col"], pc["p0"], pc["ln"]
                            nc.tensor.matmul(
                                pts[t][:],
                                vb[p0:p0 + l, col:col + 1],
                                S[p0:p0 + l, col, :],
                                start=pc["start"], stop=pc["stop"])
                    for t in s_ent["tiles"]:
                        nc.scalar.copy(stack3[0:1, t * 128:(t + 1) * 128], pts[t][:])
                    # interleaved dense for this super's tiles
                    for t in s_ent["tiles"]:
                        cols = slice(t * 128, (t + 1) * 128)
                        zt = dpsum.tile([128, 128], f32, tag="z")
                        nc.tensor.matmul(zt[:], stack3[:, cols],
                                         ws["w1stack"].ap(), start=True, stop=True)
                        ht = dsb.tile([128, 128], bf16, tag="h")
                        nc.scalar.activation(ht[:], zt[:], AF.Relu)
                        nc.sync.dma_start(h1_mine.ap()[cols, :], ht[:])
                        tp = tpsum.tile([128, 128], bf16, tag="tp")
                        nc.tensor.transpose(tp[:], ht[:], ws["ident"].ap())
                        nc.scalar.copy(h1T[:, cols], tp[:])
                    tiles_done[0] += len(s_ent["tiles"])
                    l1_quarters()
            scope_l1.__exit__(None, None, None)

        # ======================================================================
        # generic gather-aggregate + interleaved dense for layers 2/3
        # ======================================================================
        def agg_layer(tables, d_in, gidx_r, slotG_r, out_write, on_tiles,
                      gbufs, xg_stationary, xg_dt=bf16):
            """out_write(t, psum_ap): evacuate tile t's psum.
            on_tiles(tiles): dense work after a super's evacuation.

            xg_stationary=True: psum[d_in, 128dst] (lhsT=Xg) — used for L2 so
            the evac lands directly in feature-major agg2T.
            xg_stationary=False: psum[128dst, d_in] (lhsT=S) — used for L3.
            """
            elem = d_in  # bf16 elements per row
            with tc.tile_pool(name="gA", bufs=gbufs) as gpool, \
                 tc.tile_pool(name="sA", bufs=3) as spool, \
                 tc.tile_pool(name="pA", bufs=SUPERG, space="PSUM") as ppool:
                shape = [d_in, 128] if xg_stationary else [128, d_in]
                for s_ent in sched["sched_g"]:
                    pts = {}
                    for t in s_ent["tiles"]:
                        pts[t] = ppool.tile(shape, f32, tag="ps", name=f"psA_{t}")
                    for run in s_ent["runs"]:
                        b = run["b"]
                        blk_rows = NC * QROWS[b]
                        for sl in run["slices"]:
                            off, num, C = sl["off"], sl["num"], sl["ncols"]
                            xg = gpool.tile([128, C, elem], xg_dt, tag="g")
                            for e0 in range(0, num, GCAP):
                                n = min(GCAP, num - e0)
                                nc.gpsimd.dma_gather(
                                    xg[:, e0 // 128:(e0 + n) // 128, :],
                                    tables[b].ap(),
                                    gidx_r[:, (off + e0) // 16:(off + e0 + n) // 16],
                                    n, n, elem,
                                    queue_num=(off + e0) // GCAP % 4)
                            offc = off // 128
                            S = spool.tile([128, C, 128], bf16, tag="S")
                            iota_s = ws["iota128"].ap().rearrange("p f -> p () f").broadcast_to((128, C, 128))
                            slot_b = slotG_r[:, offc:offc + C].rearrange("p c -> p c ()").broadcast_to((128, C, 128))
                            nc.vector.tensor_tensor(S[:], iota_s, slot_b, ALU.is_equal)
                            for pc in sl["pieces"]:
                                t, col, p0, l = pc["t"], pc["col"], pc["p0"], pc["ln"]
                                if xg_stationary:
                                    lhsT, rhs = xg[p0:p0 + l, col, :], S[p0:p0 + l, col, :]
                                else:
                                    lhsT, rhs = S[p0:p0 + l, col, :], xg[p0:p0 + l, col, :]
                                nc.tensor.matmul(
                                    pts[t][:], lhsT, rhs,
                                    start=pc["start"], stop=pc["stop"])
                    for t in s_ent["tiles"]:
                        out_write(t, pts[t])
                    on_tiles(s_ent["tiles"])

        with tc.tile_pool(name="streamsG", bufs=1) as stpg:
            gidx_r = stpg.tile([128, LTG // 16], i16, tag="gidx")
            nc.sync.dma_start(gidx_r[:], p["gidx"].ap())
            slotG_r = stpg.tile([128, LTG // 128], bf16, tag="slotG")
            nc.sync.dma_start(slotG_r[:], p["slotG"].ap())

            # ---------------- LAYER 2 ----------------
            scope_l2 = nc.named_scope("l2"); scope_l2.__enter__()
            with tc.tile_pool(name="d2p", bufs=1, space="PSUM") as d2psum, \
                 tc.tile_pool(name="t2p", bufs=1, space="PSUM") as t2psum, \
                 tc.tile_pool(name="d2s", bufs=3) as d2sb:
                next_q2 = [0]
                tiles_done2 = [0]

                def l2_write(t, pt):
                    nc.scalar.copy(agg2T[:, t * 128:(t + 1) * 128], pt[:])

                def l2_dense(tiles):
                    for t in tiles:
                        cols = slice(t * 128, (t + 1) * 128)
                        zt = d2psum.tile([128, 256], f32, tag="z")
                        nc.tensor.matmul(zt[:], agg2T[:, cols], wsb["wrel2"].ap(), start=True, stop=False)
                        nc.tensor.matmul(zt[:], h1T[:, cols], wsb["wroot2"].ap(), start=False, stop=False)
                        nc.tensor.matmul(zt[:], ws["onesb"].ap(), wsb["b2"].ap(), start=False, stop=True)
                        ht = d2sb.tile([128, 256], bf16, tag="h")
                        nc.scalar.activation(ht[:], zt[:], AF.Relu)
                        nc.sync.dma_start(h2_mine.ap()[cols, :], ht[:])
                        for k in range(2):
                            tp = t2psum.tile([128, 128], bf16, tag="tp")
                            nc.tensor.transpose(tp[:], ht[:, k * 128:(k + 1) * 128],
                                                ws["ident"].ap())
                            dstT = h2T0 if k == 0 else h2T1
                            nc.scalar.copy(dstT.ap()[:, cols], tp[:])
                    tiles_done2[0] += len(tiles)
                    while next_q2[0] < NQ and tiles_done2[0] >= QENDT[next_q2[0]]:
                        q = next_q2[0]
                        nc.gpsimd.collective_compute(
                            "AllGather", ALU.bypass, replica_groups=groups,
                            ins=[h2_mine.ap()[QSTART[q]:QSTART[q] + QROWS[q], :].opt()],
                            outs=[h2q[q].ap().opt()])
                        next_q2[0] += 1

                agg_layer(h1q, 128, gidx_r, slotG_r, l2_write, l2_dense,
                          gbufs=6, xg_stationary=True)
            scope_l2.__exit__(None, None, None)

            # ---------------- LAYER 3 ----------------
            scope_l3 = nc.named_scope("l3"); scope_l3.__enter__()
            with tc.tile_pool(name="t3p", bufs=1, space="PSUM") as t3psum, \
                 tc.tile_pool(name="d3p", bufs=1, space="PSUM") as d3psum, \
                 tc.tile_pool(name="plp", bufs=1, space="PSUM") as plpsum, \
                 tc.tile_pool(name="t3s", bufs=4) as t3sb, \
                 tc.tile_pool(name="d3s", bufs=3) as d3sb:

                def l3_write(t, pt):
                    nc.scalar.copy(agg3[:, t, :], pt[:])

                def l3_dense(tiles):
                    for t in tiles:
                        cols = slice(t * 128, (t + 1) * 128)
                        a3T = []
                        for k in range(2):
                            tp = t3psum.tile([128, 128], bf16, tag="tp")
                            nc.tensor.transpose(tp[:], agg3[:, t, k * 128:(k + 1) * 128],
                                                ws["ident"].ap())
                            sb = t3sb.tile([128, 128], bf16, tag="a3T")
                            nc.scalar.copy(sb[:], tp[:])
                            a3T.append(sb)
                        zt = d3psum.tile([128, 512], f32, tag="z")
                        nc.tensor.matmul(zt[:], a3T[0][:], wsb["wrel3_0"].ap(), start=True, stop=False)
                        nc.tensor.matmul(zt[:], a3T[1][:], wsb["wrel3_1"].ap(), start=False, stop=False)
                        nc.tensor.matmul(zt[:], h2T0.ap()[:, cols], wsb["wroot3_0"].ap(), start=False, stop=False)
                        nc.tensor.matmul(zt[:], h2T1.ap()[:, cols], wsb["wroot3_1"].ap(), start=False, stop=False)
                        nc.tensor.matmul(zt[:], ws["onesb"].ap(), wsb["b3"].ap(), start=False, stop=True)
                        ht = d3sb.tile([128, 512], bf16, tag="h")
                        nc.scalar.copy(ht[:], zt[:])
                        # pool: B [128, G] one-hot of batch id
                        B = d3sb.tile([128, G], bf16, tag="B")
                        nc.vector.tensor_scalar(B[:], ws["iotaGb"].ap(),
                                                ws["bslot"].ap()[:, t:t + 1], None,
                                                ALU.is_equal)
                        pp = plpsum.tile([G, 512], f32, tag="pp")
                        nc.tensor.matmul(pp[:], B[:], ht[:], start=True, stop=True)
                        if t == 0:
                            nc.vector.tensor_copy(pooled_acc.ap(), pp[:])
                        else:
                            nc.vector.tensor_tensor(pooled_acc.ap(), pooled_acc.ap(),
                                                    pp[:], ALU.add)

                agg_layer(h2q, 256, gidx_r, slotG_r, l3_write, l3_dense,
                          gbufs=4, xg_stationary=False)
            scope_l3.__exit__(None, None, None)

        scope_fin = nc.named_scope("final"); scope_fin.__enter__()
        # ================= allreduce + normalize =================
        nc.sync.dma_start(pool_in.ap(), pooled_acc.ap())
        nc.gpsimd.collective_compute(
            "AllGather", ALU.bypass, replica_groups=groups,
            ins=[pool_in.ap().opt()], outs=[pool_out8.ap().opt()])
        with tc.tile_pool(name="fin", bufs=1) as fin:
            ps = fin.tile([G, 512], f32, tag="ps")
            ps8 = fin.tile([G, 8, 512], f32, tag="ps8")
            nc.sync.dma_start(
                ps8[:], pool_out8.ap().rearrange("(r g) f -> g r f", r=8))
            nc.vector.tensor_reduce(ps[:], ps8[:].rearrange("g r f -> g f r"),
                                    mybir.AxisListType.X, ALU.add)
            mean = fin.tile([G, 512], f32, tag="mean")
            nc.vector.tensor_scalar(mean[:], ps[:], ws["invcnt"].ap(), None,
                                    ALU.mult)
            sq = fin.tile([G, 512], f32, tag="sq")
            nc.vector.tensor_tensor(sq[:], mean[:], mean[:], ALU.mult)
            ss = fin.tile([G, 1], f32, tag="ss")
            nc.vector.tensor_reduce(ss[:], sq[:], mybir.AxisListType.X, ALU.add)
            nrm = fin.tile([G, 1], f32, tag="nrm")
            nc.scalar.sqrt(nrm[:], ss[:])
            nc.vector.tensor_scalar(nrm[:], nrm[:], 1e-12, None, ALU.max)
            inv = fin.tile([G, 1], f32, tag="inv")
            nc.vector.reciprocal(inv[:], nrm[:])
            outv = fin.tile([G, 512], f32, tag="outv")
            nc.vector.tensor_scalar(outv[:], mean[:], inv[:], None, ALU.mult)
            nc.sync.dma_start(out_ext.ap(), outv[:])

    scope_fin.__exit__(None, None, None)
    nc.compile()
    return nc


# ----------------------------------------------------------------------------
# In-map assembly
# ----------------------------------------------------------------------------

def make_in_maps(host, inputs, cfg):
    c = cfg
    NC = c["NC"]
    w1stack = np.concatenate([
        np.asarray(inputs["W_rel1"], np.float32).reshape(1, 128),
        np.asarray(inputs["W_root1"], np.float32).reshape(1, 128),
        np.asarray(inputs["b_rel1"], np.float32).reshape(1, 128)], axis=0).astype(BF16)
    shared = {
        "xw": host["xw"],
        "onesrow": host["onesrow"],
        "w1stack": w1stack,
        "wrel2": np.asarray(inputs["W_rel2"], np.float32),
        "wroot2": np.asarray(inputs["W_root2"], np.float32),
        "b2": np.asarray(inputs["b_rel2"], np.float32).reshape(1, 256),
        "wrel3": np.asarray(inputs["W_rel3"], np.float32),
        "wroot3": np.asarray(inputs["W_root3"], np.float32),
        "b3": np.asarray(inputs["b_rel3"], np.float32).reshape(1, 512),
        "ident": host["ident"],
        "iota128": host["iota128"],
        "iota64f": host["iota64f"],
        "iotaGb": host["iotaGb"],
        "onesb": host["onesb"],
        "invcnt": host["invcnt"],
    }
    in_maps = []
    for ci in range(NC):
        m = dict(shared)
        m["xloc"] = host["xloc"][ci]
        m["bslot"] = host["bslot"][ci]
        m["widx"] = host["widx"][ci]
        m["woff"] = host["woff"][ci]
        m["slot1"] = host["slot1"][ci]
        m["gidx"] = host["gidx"][ci]
        m["slotG"] = host["slotG"][ci]
        in_maps.append(m)
    return in_maps


# ----------------------------------------------------------------------------
# Entry points
# ----------------------------------------------------------------------------

_BUILD_CACHE = {}


def _install_ntff_shim(so_path="/opt/axon/libaxon_pjrt.so"):
    """Provide antenv.axon_hooks (absent in this image) so that
    run_bass_kernel_spmd(trace=True) can capture NTFF profiles via the
    axon PJRT plugin's C ABI."""
    import types
    import ctypes
    import contextlib

    if "antenv.axon_hooks" in sys.modules:
        return
    try:
        lib = ctypes.CDLL(so_path)
    except OSError:
        return
    if not hasattr(lib, "axon_start_nrt_profile"):
        return
    lib.axon_start_nrt_profile.argtypes = [
        ctypes.POINTER(ctypes.c_int64), ctypes.c_size_t]
    lib.axon_start_nrt_profile.restype = ctypes.c_int64
    lib.axon_stop_nrt_profile.argtypes = [ctypes.c_char_p]
    lib.axon_stop_nrt_profile.restype = ctypes.c_int64

    @contextlib.contextmanager
    def _hook(output_dir, device_ids):
        import jax
        jax.devices()
        if device_ids:
            ids = (ctypes.c_int64 * len(device_ids))(*device_ids)
            rc = lib.axon_start_nrt_profile(ids, len(device_ids))
        else:
            rc = lib.axon_start_nrt_profile(None, 0)
        if rc != 0:
            raise RuntimeError(f"axon_start_nrt_profile rc={rc}")
        try:
            yield
        finally:
            n = lib.axon_stop_nrt_profile(str(output_dir).encode())
            if n < 0:
                raise RuntimeError(f"axon_stop_nrt_profile rc={n}")
            print(f"profile: {n} file(s) written to {output_dir}")

    mod = types.ModuleType("antenv.axon_hooks")
    mod.get_axon_ntff_profile_hook = lambda: _hook
    mod.set_axon_ntff_profile_hook = lambda h: None
    sys.modules["antenv.axon_hooks"] = mod


def run(inputs, cfg=None, sim=False, trace=False, dump=False):
    cfg = derive(cfg or FULL_CFG)
    host, sched = preprocess(inputs["x"], inputs["edge_index"], inputs["batch"], cfg)
    nc = build_graph(cfg, sched, debug=sim, dump=dump)
    in_maps = make_in_maps(host, inputs, cfg)

    if sim:
        from concourse.bass_interp import MultiCoreSim
        s = MultiCoreSim(nc, num_cores=cfg["NC"])
        for ci in range(cfg["NC"]):
            for k, v in in_maps[ci].items():
                s.cores[ci].tensor(k)[:] = np.ascontiguousarray(v)
        s.simulate(check_with_hw=False)
        out = np.array(s.cores[0].mem_tensor("out"))
        return out, None
    else:
        if trace:
            _install_ntff_shim()
        from concourse import bass_utils
        res = bass_utils.run_bass_kernel_spmd(
            nc, in_maps, core_ids=list(range(cfg["NC"])), trace=trace)
        return np.asarray(res.results[0]["out"]), res


def kernel(**inputs) -> np.ndarray:
    out, _ = run(inputs, FULL_CFG, sim=False, trace=False)
    return out.astype(np.float32)



# revision 16
# speedup vs baseline: 1.2823x; 1.0887x over previous
"""Distributed Trainium2 kernel for 3-layer GraphConv GNN + global mean pool + L2 normalize.

Strategy (8 NeuronCores, SPMD):
  - Nodes sharded by contiguous ranges across cores (dst-sharding of edges).
  - Aggregation (segment_sum of gathered neighbor features) per core:
      * dma_gather pulls h[src] rows from a replicated node-feature table in HBM
        (int16 index limit handled by splitting the table into 32768-row blocks).
      * scatter side is a one-hot matmul into PSUM: for each chunk of <=128 edges,
        PSUM[tile] += onehot(dst_slot)^T-style matmul. Exact f32 accumulation.
  - Dense phase per layer on TensorEngine (bf16 operands, f32 PSUM).
  - h replicated between layers with collective AllGather (bf16).
  - Global mean-pool via batch-one-hot matmul, AllReduce of [G, 512] partials,
    then L2 normalization. All cores produce the full output.

Host-side work is strictly index preprocessing (sorting/partitioning per the
METIS-style sharding hint); no float input values are touched on host.
"""

import math
import sys

import numpy as np

sys.path.insert(0, "/opt/trn_rl_repo")

import ml_dtypes  # noqa: E402

BF16 = ml_dtypes.bfloat16

# ----------------------------------------------------------------------------
# Configs
# ----------------------------------------------------------------------------

FULL_CFG = dict(N=100000, E=800000, G=64, NC=8)
DIMS = [1, 128, 256, 512]
SUPERG = 5       # dst tiles per super-iteration, layers 2/3 (PSUM banks: 5+3 dense)
SUPER1 = 6       # dst tiles per super-iteration, layer 1 (PSUM banks: 6+2 dense)
PADQ = 64        # per-(super,quarter,tile) segment padding quantum
NQ = 4           # node-table quarters (pipelined AllGather granularity)
WIN = 64         # layer-1 f32 gather window (256B)
GCAP = 1024      # max indices per dma_gather call (one ring slot's worth)
SL = 2048        # edges per SBUF slice (gather/one-hot staging, layers 2/3)
SL1 = 2048       # edges per SBUF slice for layer 1
DMA_SCRATCH = 32768  # SWDGE descriptor carveout: 2048 descs/queue = 2 calls in flight


def derive(cfg):
    d = dict(cfg)
    N, NC = d["N"], d["NC"]
    assert N % NC == 0
    d["NPC"] = N // NC
    d["TPC"] = (d["NPC"] + 127) // 128          # node tiles per core
    d["NPC_PAD"] = d["TPC"] * 128
    d["NFULL"] = NC * d["NPC_PAD"]
    # quarter q covers tiles [QT0*q, ...): first NQ-1 quarters equal, last takes rest
    qt = (d["TPC"] + NQ - 1) // NQ
    d["QTILES"] = [qt] * (NQ - 1) + [d["TPC"] - qt * (NQ - 1)]
    d["QROWS"] = [t * 128 for t in d["QTILES"]]
    d["QSTART"] = [qt * 128 * q for q in range(NQ)]
    d["QENDT"] = [qt * (q + 1) for q in range(NQ - 1)] + [d["TPC"]]
    d["NSUP"] = (d["TPC"] + SUPERG - 1) // SUPERG
    d["NSUP1"] = (d["TPC"] + SUPER1 - 1) // SUPER1
    d["NW1"] = (d["N"] + WIN - 1) // WIN        # x windows
    return d


# ----------------------------------------------------------------------------
# Host preprocessing: edge layout + schedule
# ----------------------------------------------------------------------------

def preprocess(x, edge_index, batch, cfg):
    """Build all per-core host arrays and the static schedule."""
    c = cfg
    N, E, G, NC = c["N"], c["E"], c["G"], c["NC"]
    NPC, TPC, NPC_PAD, NFULL, NSUP, NSUP1 = (
        c["NPC"], c["TPC"], c["NPC_PAD"], c["NFULL"], c["NSUP"], c["NSUP1"])
    NBLK = NQ
    QROWS, QSTART = c["QROWS"], c["QSTART"]
    qt0 = c["QTILES"][0]

    src = np.asarray(edge_index[0], dtype=np.int64)
    dst = np.asarray(edge_index[1], dtype=np.int64)
    batch = np.asarray(batch, dtype=np.int64)

    # ---- per-core edge sets
    core_of = dst // NPC
    per_core = []
    for ci in range(NC):
        m = core_of == ci
        es, ed = src[m], dst[m] - ci * NPC
        per_core.append((es, ed))

    qrows_np = np.asarray(QROWS)
    qstart_np = np.asarray(QSTART)

    # ---- G: main gather layout (shared by layers 2 and 3)
    # order: (super, quarter, tile, src). Quarter q's table holds rows
    # [QSTART[q], QSTART[q]+QROWS[q]) of every core's shard, concatenated by
    # rank (the AllGather output layout).
    def g_keys(es, ed):
        tile = ed >> 7
        slot = ed & 127
        sup = tile // SUPERG
        cb = es // NPC
        r = es % NPC
        blk = np.minimum(r >> 7, TPC - 1) // qt0
        spad = cb * qrows_np[blk] + (r - qstart_np[blk])
        return sup, blk, tile, slot, spad

    # segment counts n[core, sup, blk, tile]
    nseg = np.zeros((NC, NSUP, NBLK, TPC), dtype=np.int64)
    gdata = []
    for ci in range(NC):
        es, ed = per_core[ci]
        sup, blk, tile, slot, spad = g_keys(es, ed)
        order = np.lexsort((spad, tile, blk, sup))
        sup, blk, tile, slot, spad = (a[order] for a in (sup, blk, tile, slot, spad))
        np.add.at(nseg[ci], (sup, blk, tile), 1)
        gdata.append((sup, blk, tile, slot, spad))

    nmax = nseg.max(axis=0)  # [NSUP, NBLK, TPC]
    npad = ((nmax + PADQ - 1) // PADQ) * PADQ
    # ensure every (sup, tile) has at least one segment so PSUM gets written
    tile_tot = npad.sum(axis=1)  # [NSUP, TPC]
    for s in range(NSUP):
        for t in range(min(TPC - s * SUPERG, SUPERG)):
            ti = s * SUPERG + t
            if ti < TPC and tile_tot[s, ti] == 0:
                npad[s, 0, ti] = PADQ

    # run = (sup, blk). run length padded to 128.
    run_len = {}
    run_off = {}   # global edge offset of run start
    seg_off = {}   # (s,b,t) -> global offset
    LT = 0
    for s in range(NSUP):
        for b in range(NBLK):
            r0 = LT
            for t in range(TPC):
                if npad[s, b, t]:
                    seg_off[(s, b, t)] = LT
                    LT += int(npad[s, b, t])
            L = LT - r0
            Lp = ((L + 127) // 128) * 128
            LT = r0 + Lp
            run_len[(s, b)] = Lp
            run_off[(s, b)] = r0
    LTG = LT

    # per-core arrays: gidx int16 (block-local padded src), slotG bf16
    gidx = np.zeros((NC, LTG), dtype=np.int16)
    slotG = np.full((NC, LTG), -1.0, dtype=np.float32)
    for ci in range(NC):
        sup, blk, tile, slot, spad = gdata[ci]
        # fill per segment
        pos = 0
        # edges are sorted by (sup, blk, tile); walk segments
        seg_ids = sup * (NBLK * TPC) + blk * TPC + tile
        bounds = np.flatnonzero(np.diff(seg_ids)) + 1
        starts = np.concatenate(([0], bounds))
        ends = np.concatenate((bounds, [len(seg_ids)]))
        for st, en in zip(starts, ends):
            s, b, t = int(sup[st]), int(blk[st]), int(tile[st])
            o = seg_off[(s, b, t)]
            n = en - st
            assert n <= npad[s, b, t]
            loc = spad[st:en]
            assert (loc >= 0).all() and (loc < NC * QROWS[b]).all()
            gidx[ci, o:o + n] = loc.astype(np.int16)
            slotG[ci, o:o + n] = slot[st:en].astype(np.float32)
            # pad entries within segment: repeat first idx (slot stays -1)
            gidx[ci, o + n: o + int(npad[s, b, t])] = loc[0] if n else 0
        del pos

    # slice-centric schedule: per s: tiles + runs; each run split into slices
    # of <= SL edges; pieces attached to the slice containing their column.
    SLC = SL // 128
    sched_g = []
    for s in range(NSUP):
        tiles = list(range(s * SUPERG, min((s + 1) * SUPERG, TPC)))
        # pieces per tile in edge order, with start/stop flags
        runs = []
        for b in range(NBLK):
            L = run_len[(s, b)]
            if not L:
                continue
            ncols = L // 128
            slices = []
            for c0 in range(0, ncols, SLC):
                nc_ = min(SLC, ncols - c0)
                slices.append(dict(c0=c0, ncols=nc_,
                                   off=run_off[(s, b)] + c0 * 128,
                                   num=nc_ * 128, pieces=[]))
            runs.append(dict(b=b, off=run_off[(s, b)], num=L, slices=slices))
        run_by_b = {r["b"]: r for r in runs}
        for t in tiles:
            pieces = []
            for b in range(NBLK):
                if (s, b, t) not in seg_off:
                    continue
                o = seg_off[(s, b, t)]
                ln_tot = int(npad[s, b, t])
                lo = o - run_off[(s, b)]
                while ln_tot > 0:
                    p0 = lo % 128
                    cap = 128 if p0 == 0 else (64 if p0 == 64 else 32)
                    l = min(ln_tot, cap)
                    pieces.append((b, lo // 128, p0, l))
                    lo += l
                    ln_tot -= l
            assert pieces
            for i, (b, col, p0, l) in enumerate(pieces):
                sl = run_by_b[b]["slices"][col // SLC]
                sl["pieces"].append(dict(
                    t=t, col=col - sl["c0"], p0=p0, ln=l,
                    start=(i == 0), stop=(i == len(pieces) - 1)))
        sched_g.append(dict(tiles=tiles, runs=runs))

    # ---- W: layer-1 gather layout: order (super, tile, src)
    def w_keys(es, ed):
        tile = ed >> 7
        slot = ed & 127
        sup = tile // SUPER1
        win = es // WIN
        off = es % WIN
        return sup, tile, slot, win, off

    nseg1 = np.zeros((NC, NSUP1, TPC), dtype=np.int64)
    wdata = []
    for ci in range(NC):
        es, ed = per_core[ci]
        sup, tile, slot, win, off = w_keys(es, ed)
        order = np.lexsort((win, tile, sup))
        sup, tile, slot, win, off = (a[order] for a in (sup, tile, slot, win, off))
        np.add.at(nseg1[ci], (sup, tile), 1)
        wdata.append((sup, tile, slot, win, off))

    nmax1 = nseg1.max(axis=0)
    npad1 = ((nmax1 + PADQ - 1) // PADQ) * PADQ
    for s in range(NSUP1):
        for t in range(s * SUPER1, min((s + 1) * SUPER1, TPC)):
            if npad1[s, t] == 0:
                npad1[s, t] = PADQ

    seg_off1 = {}
    run_len1 = {}
    run_off1 = {}
    LT = 0
    for s in range(NSUP1):
        r0 = LT
        for t in range(TPC):
            if npad1[s, t]:
                seg_off1[(s, t)] = LT
                LT += int(npad1[s, t])
        L = LT - r0
        Lp = ((L + 127) // 128) * 128
        LT = r0 + Lp
        run_len1[s] = Lp
        run_off1[s] = r0
    LT1 = LT

    widx = np.zeros((NC, LT1), dtype=np.int16)
    woff = np.full((NC, LT1), -1.0, dtype=np.float32)
    slot1 = np.full((NC, LT1), -1.0, dtype=np.float32)
    for ci in range(NC):
        sup, tile, slot, win, off = wdata[ci]
        seg_ids = sup * TPC + tile
        bounds = np.flatnonzero(np.diff(seg_ids)) + 1
        starts = np.concatenate(([0], bounds))
        ends = np.concatenate((bounds, [len(seg_ids)]))
        for st, en in zip(starts, ends):
            s, t = int(sup[st]), int(tile[st])
            o = seg_off1[(s, t)]
            n = en - st
            widx[ci, o:o + n] = win[st:en].astype(np.int16)
            woff[ci, o:o + n] = off[st:en].astype(np.float32)
            slot1[ci, o:o + n] = slot[st:en].astype(np.float32)
            widx[ci, o + n:o + int(npad1[s, t])] = win[0] if n else 0

    SLC1 = SL1 // 128
    sched_1 = []
    for s in range(NSUP1):
        tiles = list(range(s * SUPER1, min((s + 1) * SUPER1, TPC)))
        L = run_len1[s]
        ncols = L // 128
        slices = []
        for c0 in range(0, ncols, SLC1):
            nc_ = min(SLC1, ncols - c0)
            slices.append(dict(c0=c0, ncols=nc_, off=run_off1[s] + c0 * 128,
                               num=nc_ * 128, pieces=[]))
        for t in tiles:
            pieces = []
            if (s, t) in seg_off1:
                o = seg_off1[(s, t)]
                ln_tot = int(npad1[s, t])
                lo = o - run_off1[s]
                while ln_tot > 0:
                    p0 = lo % 128
                    cap = 128 if p0 == 0 else (64 if p0 == 64 else 32)
                    l = min(ln_tot, cap)
                    pieces.append((lo // 128, p0, l))
                    lo += l
                    ln_tot -= l
            assert pieces
            for i, (col, p0, l) in enumerate(pieces):
                sl = slices[col // SLC1]
                sl["pieces"].append(dict(
                    t=t, col=col - sl["c0"], p0=p0, ln=l,
                    start=(i == 0), stop=(i == len(pieces) - 1)))
        sched_1.append(dict(tiles=tiles, slices=slices))

    # ---- idx wrap helper: entry i -> [i%16 (+16g), i//16], replicated 8 groups
    def wrap16(a):
        # a: [NC, L] -> [NC, 128, L//16]
        L = a.shape[1]
        assert L % 16 == 0
        w = a.reshape(a.shape[0], L // 16, 16).transpose(0, 2, 1)  # [NC,16,L/16]
        return np.tile(w, (1, 8, 1)).copy()

    def wrap128(a, dtype):
        L = a.shape[1]
        assert L % 128 == 0
        return a.reshape(a.shape[0], L // 128, 128).transpose(0, 2, 1).astype(dtype).copy()

    host = {}
    host["gidx"] = wrap16(gidx)                       # [NC,128,LTG/16] i16
    host["slotG"] = wrap128(slotG, BF16)              # [NC,128,LTG/128]
    host["widx"] = wrap16(widx)                       # [NC,128,LT1/16]
    host["woff"] = wrap128(woff, np.float32)
    host["slot1"] = wrap128(slot1, BF16)

    # ---- x windows, x local, batch slots, counts
    xf = np.asarray(x, dtype=np.float32).reshape(-1)
    xw = np.zeros((c["NW1"] * WIN,), dtype=np.float32)
    xw[:N] = xf
    host["xw"] = xw.reshape(c["NW1"], WIN)

    xloc = np.zeros((NC, 1, NPC_PAD), dtype=np.float32)
    bslot = np.full((NC, NPC_PAD), -1.0, dtype=np.float32)
    for ci in range(NC):
        xloc[ci, 0, :NPC] = xf[ci * NPC:(ci + 1) * NPC]
        bslot[ci, :NPC] = batch[ci * NPC:(ci + 1) * NPC].astype(np.float32)
    host["xloc"] = xloc.astype(BF16)
    # bslot as [128, TPC]: node 128*t+p -> [p, t]
    host["bslot"] = bslot.reshape(NC, TPC, 128).transpose(0, 2, 1).astype(np.float32).copy()

    counts = np.bincount(batch, minlength=G).astype(np.float64)
    host["invcnt"] = (1.0 / np.maximum(counts, 1.0)).astype(np.float32).reshape(G, 1)

    host["onesrow"] = np.ones((1, NPC_PAD), dtype=BF16)
    host["ident"] = np.eye(128, dtype=np.float32).astype(BF16)
    host["iota128"] = np.broadcast_to(
        np.arange(128, dtype=np.float32), (128, 128)).astype(BF16).copy()
    host["iota64f"] = np.broadcast_to(
        np.arange(WIN, dtype=np.float32), (128, WIN)).copy()
    host["iotaGb"] = np.broadcast_to(
        np.arange(G, dtype=np.float32), (128, G)).astype(BF16).copy()
    host["onesb"] = np.ones((1, 128), dtype=np.float32).astype(BF16)

    sched = dict(sched_g=sched_g, sched_1=sched_1, LTG=LTG, LT1=LT1)
    return host, sched


# ----------------------------------------------------------------------------
# Graph builder
# ----------------------------------------------------------------------------

def build_graph(cfg, sched, debug=False, dump=False):
    from concourse import bass, bacc, tile, mybir

    c = cfg
    G = c["G"]
    NC = c["NC"]
    NPC_PAD, NFULL, TPC = c["NPC_PAD"], c["NFULL"], c["TPC"]
    QROWS, QSTART, QENDT = c["QROWS"], c["QSTART"], c["QENDT"]
    f32 = mybir.dt.float32
    bf16 = mybir.dt.bfloat16
    fp8 = mybir.dt.float8e4
    i16 = mybir.dt.int16
    AF = mybir.ActivationFunctionType
    ALU = mybir.AluOpType

    LTG, LT1 = sched["LTG"], sched["LT1"]

    nc = bacc.Bacc("TRN2", target_bir_lowering=False, debug=debug,
                   num_devices=NC, num_swdge_queues=4,
                   dynamic_dma_scratch_size=DMA_SCRATCH)

    # ---------------- dram parameters ----------------
    def din(name, shape, dtype):
        return nc.dram_tensor(name, list(shape), dtype, kind="ExternalInput")

    p = {}
    p["xw"] = din("xw", (c["NW1"], WIN), f32)
    p["xloc"] = din("xloc", (1, NPC_PAD), bf16)
    p["onesrow"] = din("onesrow", (1, NPC_PAD), bf16)
    p["w1stack"] = din("w1stack", (3, 128), bf16)
    p["wrel2"] = din("wrel2", (128, 256), f32)
    p["wroot2"] = din("wroot2", (128, 256), f32)
    p["b2"] = din("b2", (1, 256), f32)
    p["wrel3"] = din("wrel3", (256, 512), f32)
    p["wroot3"] = din("wroot3", (256, 512), f32)
    p["b3rep"] = din("b3rep", (G, 512), f32)
    p["ident"] = din("ident", (128, 128), bf16)
    p["iota128"] = din("iota128", (128, 128), bf16)
    p["iota64f"] = din("iota64f", (128, WIN), f32)
    p["iotaGb"] = din("iotaGb", (128, G), bf16)
    p["onesb"] = din("onesb", (1, 128), bf16)
    p["invcnt"] = din("invcnt", (G, 1), f32)
    p["bslot"] = din("bslot", (128, TPC), f32)
    p["widx"] = din("widx", (128, LT1 // 16), i16)
    p["woff"] = din("woff", (128, LT1 // 128), f32)
    p["slot1"] = din("slot1", (128, LT1 // 128), bf16)
    p["gidx"] = din("gidx", (128, LTG // 16), i16)
    p["slotG"] = din("slotG", (128, LTG // 128), bf16)

    out_ext = nc.dram_tensor("out", [G, 512], f32, kind="ExternalOutput")

    # internal dram
    h1_mine = nc.dram_tensor("h1_mine", [NPC_PAD, 128], bf16)
    h2_mine = nc.dram_tensor("h2_mine", [NPC_PAD, 256], fp8)
    h1q = [nc.dram_tensor(f"h1q{q}", [NC * QROWS[q], 128], bf16,
                          addr_space="Shared") for q in range(NQ)]
    h2q = [nc.dram_tensor(f"h2q{q}", [NC * QROWS[q], 256], fp8,
                          addr_space="Shared") for q in range(NQ)]
    pool_in = nc.dram_tensor("pool_in", [G, 512], f32)
    pool_red = nc.dram_tensor("pool_red", [G, 512], f32, addr_space="Shared")

    # ---------------- persistent sbuf ----------------
    # arena: h1T / agg2T during L1-L2; agg3 (node-major [128, TPC*256]) in L3
    arena = nc.alloc_sbuf_tensor("arena", [128, 2 * NPC_PAD], bf16)
    h1T = arena.ap()[:, 0:NPC_PAD]
    agg2T = arena.ap()[:, NPC_PAD:2 * NPC_PAD]
    agg3 = arena.ap().rearrange("p (t d) -> p t d", d=256)  # [128, ..., 256]

    h2T0 = nc.alloc_sbuf_tensor("h2T0", [128, NPC_PAD], bf16)
    h2T1 = nc.alloc_sbuf_tensor("h2T1", [128, NPC_PAD], bf16)
    pooled_acc = nc.alloc_sbuf_tensor("pooled_acc", [G, 512], f32)

    ws = {}
    for name, shape, dt_ in [
        ("w1stack", (3, 128), bf16), ("ident", (128, 128), bf16),
        ("iota128", (128, 128), bf16), ("iota64f", (128, WIN), f32),
        ("iotaGb", (128, G), bf16), ("onesb", (1, 128), bf16),
        ("invcnt", (G, 1), f32), ("bslot", (128, TPC), f32),
        ("b3rep", (G, 512), f32),
    ]:
        ws[name] = nc.alloc_sbuf_tensor("sb_" + name, list(shape), dt_)
    # bf16 weights
    wsb = {}
    for name, shape in [("wrel2", (128, 256)), ("wroot2", (128, 256)),
                        ("b2", (1, 256))]:
        wsb[name] = nc.alloc_sbuf_tensor("sbb_" + name, list(shape), bf16)
    for name in ("wrel3", "wroot3"):
        wsb[name + "_0"] = nc.alloc_sbuf_tensor("sbb_" + name + "_0", [128, 512], bf16)
        wsb[name + "_1"] = nc.alloc_sbuf_tensor("sbb_" + name + "_1", [128, 512], bf16)

    groups = [list(range(NC))]

    with tile.TileContext(nc) as tc:
        # ---------------- load constants ----------------
        with tc.tile_pool(name="wtmp", bufs=2) as wtmp:
            for name in ("w1stack", "ident", "iota128", "iota64f", "iotaGb",
                         "onesb", "invcnt", "bslot", "b3rep"):
                nc.sync.dma_start(ws[name].ap(), p[name].ap())
            for name in ("wrel2", "wroot2", "b2"):
                t = wtmp.tile(list(p[name].shape), f32, tag="wtmp")
                nc.sync.dma_start(t[:], p[name].ap())
                nc.scalar.copy(wsb[name].ap(), t[:])
            for name in ("wrel3", "wroot3"):
                for k in range(2):
                    t = wtmp.tile([128, 512], f32, tag="wtmp3")
                    nc.sync.dma_start(t[:], p[name].ap()[k * 128:(k + 1) * 128, :])
                    nc.scalar.copy(wsb[name + f"_{k}"].ap(), t[:])

        # ======================================================================
        # LAYER 1: gather-aggregate + interleaved dense + quarter AllGathers
        # ======================================================================
        with tc.tile_pool(name="streams1", bufs=1) as stp1, \
             tc.tile_pool(name="stack3p", bufs=1) as s3p:
            widx_r = stp1.tile([128, LT1 // 16], i16, tag="widx")
            nc.sync.dma_start(widx_r[:], p["widx"].ap())
            woff_r = stp1.tile([128, LT1 // 128], f32, tag="woff")
            nc.sync.dma_start(woff_r[:], p["woff"].ap())
            slot1_r = stp1.tile([128, LT1 // 128], bf16, tag="slot1")
            nc.sync.dma_start(slot1_r[:], p["slot1"].ap())
            stack3 = s3p.tile([3, NPC_PAD], bf16, tag="stack3")
            nc.sync.dma_start(stack3[1:2, :], p["xloc"].ap())
            nc.sync.dma_start(stack3[2:3, :], p["onesrow"].ap())

            scope_l1 = nc.named_scope("l1"); scope_l1.__enter__()
            with tc.tile_pool(name="g1", bufs=3) as gpool, \
                 tc.tile_pool(name="s1", bufs=3) as spool, \
                 tc.tile_pool(name="m1", bufs=3) as mpool, \
                 tc.tile_pool(name="p1", bufs=SUPER1, space="PSUM") as ppool, \
                 tc.tile_pool(name="d1p", bufs=1, space="PSUM") as dpsum, \
                 tc.tile_pool(name="t1p", bufs=1, space="PSUM") as tpsum, \
                 tc.tile_pool(name="d1s", bufs=2) as dsb:
                next_q = [0]
                tiles_done = [0]

                def l1_quarters():
                    while next_q[0] < NQ and tiles_done[0] >= QENDT[next_q[0]]:
                        q = next_q[0]
                        nc.gpsimd.collective_compute(
                            "AllGather", ALU.bypass, replica_groups=groups,
                            ins=[h1_mine.ap()[QSTART[q]:QSTART[q] + QROWS[q], :].opt()],
                            outs=[h1q[q].ap().opt()])
                        next_q[0] += 1

                for s_ent in sched["sched_1"]:
                    pts = {}
                    for t in s_ent["tiles"]:
                        pts[t] = ppool.tile([1, 128], f32, tag="ps", name=f"ps1_{t}")
                    for sl in s_ent["slices"]:
                        off, num, C = sl["off"], sl["num"], sl["ncols"]
                        xg = gpool.tile([128, C, WIN], f32, tag="g")
                        for e0 in range(0, num, GCAP):
                            n = min(GCAP, num - e0)
                            nc.gpsimd.dma_gather(
                                xg[:, e0 // 128:(e0 + n) // 128, :], p["xw"].ap(),
                                widx_r[:, (off + e0) // 16:(off + e0 + n) // 16],
                                n, n, WIN,
                                queue_num=(off + e0) // GCAP % 4)
                        offc = off // 128
                        # mask / v
                        mask = mpool.tile([128, C, WIN], f32, tag="mask")
                        iota_b = ws["iota64f"].ap().rearrange("p w -> p () w").broadcast_to((128, C, WIN))
                        woff_b = woff_r[:, offc:offc + C].rearrange("p c -> p c ()").broadcast_to((128, C, WIN))
                        nc.vector.tensor_tensor(mask[:], iota_b, woff_b, ALU.is_equal)
                        nc.vector.tensor_tensor(mask[:], mask[:], xg[:], ALU.mult)
                        vf = mpool.tile([128, C], f32, tag="vf")
                        nc.vector.tensor_reduce(vf[:], mask[:], mybir.AxisListType.X, ALU.add)
                        vb = mpool.tile([128, C], bf16, tag="vb")
                        nc.scalar.copy(vb[:], vf[:])
                        # S one-hot
                        S = spool.tile([128, C, 128], bf16, tag="S")
                        iota_s = ws["iota128"].ap().rearrange("p f -> p () f").broadcast_to((128, C, 128))
                        slot_b = slot1_r[:, offc:offc + C].rearrange("p c -> p c ()").broadcast_to((128, C, 128))
                        nc.vector.tensor_tensor(S[:], iota_s, slot_b, ALU.is_equal)
                        for pc in sl["pieces"]:
                            t, col, p0, l = pc["t"], pc["col"], pc["p0"], pc["ln"]
                            nc.tensor.matmul(
                                pts[t][:],
                                vb[p0:p0 + l, col:col + 1],
                                S[p0:p0 + l, col, :],
                                start=pc["start"], stop=pc["stop"])
                    for t in s_ent["tiles"]:
                        nc.scalar.copy(stack3[0:1, t * 128:(t + 1) * 128], pts[t][:])
                    # interleaved dense for this super's tiles
                    for t in s_ent["tiles"]:
                        cols = slice(t * 128, (t + 1) * 128)
                        zt = dpsum.tile([128, 128], f32, tag="z")
                        nc.tensor.matmul(zt[:], stack3[:, cols],
                                         ws["w1stack"].ap(), start=True, stop=True)
                        ht = dsb.tile([128, 128], bf16, tag="h")
                        nc.scalar.activation(ht[:], zt[:], AF.Relu)
                        nc.sync.dma_start(h1_mine.ap()[cols, :], ht[:])
                        tp = tpsum.tile([128, 128], bf16, tag="tp")
                        nc.tensor.transpose(tp[:], ht[:], ws["ident"].ap())
                        nc.scalar.copy(h1T[:, cols], tp[:])
                    tiles_done[0] += len(s_ent["tiles"])
                    l1_quarters()
            scope_l1.__exit__(None, None, None)

        # ======================================================================
        # generic gather-aggregate + interleaved dense for layers 2/3
        # ======================================================================
        def agg_layer(tables, d_in, gidx_r, slotG_r, out_write, on_tiles,
                      gbufs, xg_stationary, xg_dt=bf16):
            """out_write(t, psum_ap): evacuate tile t's psum.
            on_tiles(tiles): dense work after a super's evacuation.

            xg_stationary=True: psum[d_in, 128dst] (lhsT=Xg) — used for L2 so
            the evac lands directly in feature-major agg2T.
            xg_stationary=False: psum[128dst, d_in] (lhsT=S) — used for L3.
            """
            elem = d_in  # bf16 elements per row
            with tc.tile_pool(name="gA", bufs=gbufs) as gpool, \
                 tc.tile_pool(name="sA", bufs=3) as spool, \
                 tc.tile_pool(name="pA", bufs=SUPERG, space="PSUM") as ppool:
                shape = [d_in, 128] if xg_stationary else [128, d_in]
                for s_ent in sched["sched_g"]:
                    pts = {}
                    for t in s_ent["tiles"]:
                        pts[t] = ppool.tile(shape, f32, tag="ps", name=f"psA_{t}")
                    for run in s_ent["runs"]:
                        b = run["b"]
                        blk_rows = NC * QROWS[b]
                        for sl in run["slices"]:
                            off, num, C = sl["off"], sl["num"], sl["ncols"]
                            xg = gpool.tile([128, C, elem], xg_dt, tag="g")
                            for e0 in range(0, num, GCAP):
                                n = min(GCAP, num - e0)
                                nc.gpsimd.dma_gather(
                                    xg[:, e0 // 128:(e0 + n) // 128, :],
                                    tables[b].ap(),
                                    gidx_r[:, (off + e0) // 16:(off + e0 + n) // 16],
                                    n, n, elem,
                                    queue_num=(off + e0) // GCAP % 4)
                            offc = off // 128
                            S = spool.tile([128, C, 128], xg_dt, tag="S")
                            iota_s = ws["iota128"].ap().rearrange("p f -> p () f").broadcast_to((128, C, 128))
                            slot_b = slotG_r[:, offc:offc + C].rearrange("p c -> p c ()").broadcast_to((128, C, 128))
                            nc.vector.tensor_tensor(S[:], iota_s, slot_b, ALU.is_equal)
                            for pc in sl["pieces"]:
                                t, col, p0, l = pc["t"], pc["col"], pc["p0"], pc["ln"]
                                if xg_stationary:
                                    lhsT, rhs = xg[p0:p0 + l, col, :], S[p0:p0 + l, col, :]
                                else:
                                    lhsT, rhs = S[p0:p0 + l, col, :], xg[p0:p0 + l, col, :]
                                nc.tensor.matmul(
                                    pts[t][:], lhsT, rhs,
                                    start=pc["start"], stop=pc["stop"])
                    for t in s_ent["tiles"]:
                        out_write(t, pts[t])
                    on_tiles(s_ent["tiles"])

        with tc.tile_pool(name="streamsG", bufs=1) as stpg:
            gidx_r = stpg.tile([128, LTG // 16], i16, tag="gidx")
            nc.sync.dma_start(gidx_r[:], p["gidx"].ap())
            slotG_r = stpg.tile([128, LTG // 128], bf16, tag="slotG")
            nc.sync.dma_start(slotG_r[:], p["slotG"].ap())

            # ---------------- LAYER 2 ----------------
            scope_l2 = nc.named_scope("l2"); scope_l2.__enter__()
            with tc.tile_pool(name="d2p", bufs=1, space="PSUM") as d2psum, \
                 tc.tile_pool(name="t2p", bufs=1, space="PSUM") as t2psum, \
                 tc.tile_pool(name="d2s", bufs=3) as d2sb:
                next_q2 = [0]
                tiles_done2 = [0]

                def l2_write(t, pt):
                    nc.scalar.copy(agg2T[:, t * 128:(t + 1) * 128], pt[:])

                def l2_dense(tiles):
                    for t in tiles:
                        cols = slice(t * 128, (t + 1) * 128)
                        zt = d2psum.tile([128, 256], f32, tag="z")
                        nc.tensor.matmul(zt[:], agg2T[:, cols], wsb["wrel2"].ap(), start=True, stop=False)
                        nc.tensor.matmul(zt[:], h1T[:, cols], wsb["wroot2"].ap(), start=False, stop=False)
                        nc.tensor.matmul(zt[:], ws["onesb"].ap(), wsb["b2"].ap(), start=False, stop=True)
                        ht = d2sb.tile([128, 256], bf16, tag="h")
                        nc.scalar.activation(ht[:], zt[:], AF.Relu)
                        ht8 = d2sb.tile([128, 256], fp8, tag="h8")
                        nc.scalar.copy(ht8[:], ht[:])
                        nc.sync.dma_start(h2_mine.ap()[cols, :], ht8[:])
                        for k in range(2):
                            tp = t2psum.tile([128, 128], bf16, tag="tp")
                            nc.tensor.transpose(tp[:], ht[:, k * 128:(k + 1) * 128],
                                                ws["ident"].ap())
                            dstT = h2T0 if k == 0 else h2T1
                            nc.scalar.copy(dstT.ap()[:, cols], tp[:])
                    tiles_done2[0] += len(tiles)
                    while next_q2[0] < NQ and tiles_done2[0] >= QENDT[next_q2[0]]:
                        q = next_q2[0]
                        nc.gpsimd.collective_compute(
                            "AllGather", ALU.bypass, replica_groups=groups,
                            ins=[h2_mine.ap()[QSTART[q]:QSTART[q] + QROWS[q], :].opt()],
                            outs=[h2q[q].ap().opt()])
                        next_q2[0] += 1

                agg_layer(h1q, 128, gidx_r, slotG_r, l2_write, l2_dense,
                          gbufs=6, xg_stationary=True)
            scope_l2.__exit__(None, None, None)

            # ---------------- LAYER 3 ----------------
            scope_l3 = nc.named_scope("l3"); scope_l3.__enter__()
            with tc.tile_pool(name="t3p", bufs=1, space="PSUM") as t3psum, \
                 tc.tile_pool(name="d3p", bufs=1, space="PSUM") as d3psum, \
                 tc.tile_pool(name="plp", bufs=1, space="PSUM") as plpsum, \
                 tc.tile_pool(name="t3s", bufs=4) as t3sb, \
                 tc.tile_pool(name="d3s", bufs=3) as d3sb:

                def l3_write(t, pt):
                    nc.scalar.copy(agg3[:, t, :], pt[:])

                def l3_dense(tiles):
                    for t in tiles:
                        cols = slice(t * 128, (t + 1) * 128)
                        a3T = []
                        for k in range(2):
                            tp = t3psum.tile([128, 128], bf16, tag="tp")
                            nc.tensor.transpose(tp[:], agg3[:, t, k * 128:(k + 1) * 128],
                                                ws["ident"].ap())
                            sb = t3sb.tile([128, 128], bf16, tag="a3T")
                            nc.scalar.copy(sb[:], tp[:])
                            a3T.append(sb)
                        zt = d3psum.tile([128, 512], f32, tag="z")
                        nc.tensor.matmul(zt[:], a3T[0][:], wsb["wrel3_0"].ap(), start=True, stop=False)
                        nc.tensor.matmul(zt[:], a3T[1][:], wsb["wrel3_1"].ap(), start=False, stop=False)
                        nc.tensor.matmul(zt[:], h2T0.ap()[:, cols], wsb["wroot3_0"].ap(), start=False, stop=False)
                        nc.tensor.matmul(zt[:], h2T1.ap()[:, cols], wsb["wroot3_1"].ap(), start=False, stop=True)
                        ht = d3sb.tile([128, 512], bf16, tag="h")
                        nc.scalar.copy(ht[:], zt[:])
                        # pool: B [128, G] one-hot of batch id
                        B = d3sb.tile([128, G], bf16, tag="B")
                        nc.vector.tensor_scalar(B[:], ws["iotaGb"].ap(),
                                                ws["bslot"].ap()[:, t:t + 1], None,
                                                ALU.is_equal)
                        pp = plpsum.tile([G, 512], f32, tag="pp")
                        nc.tensor.matmul(pp[:], B[:], ht[:], start=True, stop=True)
                        if t == 0:
                            nc.vector.tensor_copy(pooled_acc.ap(), pp[:])
                        else:
                            nc.vector.tensor_tensor(pooled_acc.ap(), pooled_acc.ap(),
                                                    pp[:], ALU.add)

                agg_layer(h2q, 256, gidx_r, slotG_r, l3_write, l3_dense,
                          gbufs=6, xg_stationary=False, xg_dt=fp8)
            scope_l3.__exit__(None, None, None)

        scope_fin = nc.named_scope("final"); scope_fin.__enter__()
        # ================= allreduce + normalize =================
        nc.sync.dma_start(pool_in.ap(), pooled_acc.ap())
        nc.gpsimd.collective_compute(
            "AllReduce", ALU.add, replica_groups=groups,
            ins=[pool_in.ap().opt()], outs=[pool_red.ap().opt()])
        with tc.tile_pool(name="fin", bufs=1) as fin:
            ps = fin.tile([G, 512], f32, tag="ps")
            nc.sync.dma_start(ps[:], pool_red.ap())
            mean = fin.tile([G, 512], f32, tag="mean")
            nc.vector.tensor_scalar(mean[:], ps[:], ws["invcnt"].ap(), None,
                                    ALU.mult)
            nc.vector.tensor_tensor(mean[:], mean[:], ws["b3rep"].ap(), ALU.add)
            sq = fin.tile([G, 512], f32, tag="sq")
            nc.vector.tensor_tensor(sq[:], mean[:], mean[:], ALU.mult)
            ss = fin.tile([G, 1], f32, tag="ss")
            nc.vector.tensor_reduce(ss[:], sq[:], mybir.AxisListType.X, ALU.add)
            nrm = fin.tile([G, 1], f32, tag="nrm")
            nc.scalar.sqrt(nrm[:], ss[:])
            nc.vector.tensor_scalar(nrm[:], nrm[:], 1e-12, None, ALU.max)
            inv = fin.tile([G, 1], f32, tag="inv")
            nc.vector.reciprocal(inv[:], nrm[:])
            outv = fin.tile([G, 512], f32, tag="outv")
            nc.vector.tensor_scalar(outv[:], mean[:], inv[:], None, ALU.mult)
            nc.sync.dma_start(out_ext.ap(), outv[:])

    scope_fin.__exit__(None, None, None)
    nc.compile()
    return nc


# ----------------------------------------------------------------------------
# In-map assembly
# ----------------------------------------------------------------------------

def make_in_maps(host, inputs, cfg):
    c = cfg
    NC = c["NC"]
    w1stack = np.concatenate([
        np.asarray(inputs["W_rel1"], np.float32).reshape(1, 128),
        np.asarray(inputs["W_root1"], np.float32).reshape(1, 128),
        np.asarray(inputs["b_rel1"], np.float32).reshape(1, 128)], axis=0).astype(BF16)
    shared = {
        "xw": host["xw"],
        "onesrow": host["onesrow"],
        "w1stack": w1stack,
        "wrel2": np.asarray(inputs["W_rel2"], np.float32),
        "wroot2": np.asarray(inputs["W_root2"], np.float32),
        "b2": np.asarray(inputs["b_rel2"], np.float32).reshape(1, 256),
        "wrel3": np.asarray(inputs["W_rel3"], np.float32),
        "wroot3": np.asarray(inputs["W_root3"], np.float32),
        "b3rep": np.tile(np.asarray(inputs["b_rel3"], np.float32).reshape(1, 512), (64, 1)),
        "ident": host["ident"],
        "iota128": host["iota128"],
        "iota64f": host["iota64f"],
        "iotaGb": host["iotaGb"],
        "onesb": host["onesb"],
        "invcnt": host["invcnt"],
    }
    in_maps = []
    for ci in range(NC):
        m = dict(shared)
        m["xloc"] = host["xloc"][ci]
        m["bslot"] = host["bslot"][ci]
        m["widx"] = host["widx"][ci]
        m["woff"] = host["woff"][ci]
        m["slot1"] = host["slot1"][ci]
        m["gidx"] = host["gidx"][ci]
        m["slotG"] = host["slotG"][ci]
        in_maps.append(m)
    return in_maps


# ----------------------------------------------------------------------------
# Entry points
# ----------------------------------------------------------------------------

_BUILD_CACHE = {}


def _install_ntff_shim(so_path="/opt/axon/libaxon_pjrt.so"):
    """Provide antenv.axon_hooks (absent in this image) so that
    run_bass_kernel_spmd(trace=True) can capture NTFF profiles via the
    axon PJRT plugin's C ABI."""
    import types
    import ctypes
    import contextlib

    if "antenv.axon_hooks" in sys.modules:
        return
    try:
        lib = ctypes.CDLL(so_path)
    except OSError:
        return
    if not hasattr(lib, "axon_start_nrt_profile"):
        return
    lib.axon_start_nrt_profile.argtypes = [
        ctypes.POINTER(ctypes.c_int64), ctypes.c_size_t]
    lib.axon_start_nrt_profile.restype = ctypes.c_int64
    lib.axon_stop_nrt_profile.argtypes = [ctypes.c_char_p]
    lib.axon_stop_nrt_profile.restype = ctypes.c_int64

    @contextlib.contextmanager
    def _hook(output_dir, device_ids):
        import jax
        jax.devices()
        if device_ids:
            ids = (ctypes.c_int64 * len(device_ids))(*device_ids)
            rc = lib.axon_start_nrt_profile(ids, len(device_ids))
        else:
            rc = lib.axon_start_nrt_profile(None, 0)
        if rc != 0:
            raise RuntimeError(f"axon_start_nrt_profile rc={rc}")
        try:
            yield
        finally:
            n = lib.axon_stop_nrt_profile(str(output_dir).encode())
            if n < 0:
                raise RuntimeError(f"axon_stop_nrt_profile rc={n}")
            print(f"profile: {n} file(s) written to {output_dir}")

    mod = types.ModuleType("antenv.axon_hooks")
    mod.get_axon_ntff_profile_hook = lambda: _hook
    mod.set_axon_ntff_profile_hook = lambda h: None
    sys.modules["antenv.axon_hooks"] = mod


def run(inputs, cfg=None, sim=False, trace=False, dump=False):
    cfg = derive(cfg or FULL_CFG)
    host, sched = preprocess(inputs["x"], inputs["edge_index"], inputs["batch"], cfg)
    nc = build_graph(cfg, sched, debug=sim, dump=dump)
    in_maps = make_in_maps(host, inputs, cfg)

    if sim:
        from concourse.bass_interp import MultiCoreSim
        s = MultiCoreSim(nc, num_cores=cfg["NC"])
        for ci in range(cfg["NC"]):
            for k, v in in_maps[ci].items():
                s.cores[ci].tensor(k)[:] = np.ascontiguousarray(v)
        s.simulate(check_with_hw=False)
        out = np.array(s.cores[0].mem_tensor("out"))
        return out, None
    else:
        if trace:
            _install_ntff_shim()
        from concourse import bass_utils
        res = bass_utils.run_bass_kernel_spmd(
            nc, in_maps, core_ids=list(range(cfg["NC"])), trace=trace)
        return np.asarray(res.results[0]["out"]), res


def kernel(**inputs) -> np.ndarray:
    out, _ = run(inputs, FULL_CFG, sim=False, trace=False)
    return out.astype(np.float32)

